# revision 1
# baseline (speedup 1.0000x reference)
"""Contrastive loss kernel for Trainium2 (8 NeuronCores, Bass/Tile).

Strategy
--------
Only rows with label==1 (pos) contribute losses, and only columns with
label==0 (neg) plus the diagonal enter each row's logsumexp.  The host
computes the tiny index sets from `labels`, then each of the 8 cores
(2 per batch) receives:
  gp: its half of the batch's positive greek rows      [P1, 256] f32
  ep: english rows at the same indices (for the diag)  [P1, 256] f32
  en: all negative english rows of the batch           [N1, 256] f32
padded with zero rows to the uniform compile-time shapes (P1, N1).

On device: L2-normalize rows (1/temperature folded into the greek
scale), cast bf16, PE-transpose to put H on partitions, matmul to get
logits in PSUM, then a single fused ScalarE pass exp(logit - 15) with
accumulate gives the per-row negative sums.  A fixed max constant (15 >
1/0.07) replaces the per-row max: logits are bounded so the logsumexp
stays exact in f32.  Zero-padded `en` rows yield *exactly* 0 logits, so
their exp(-15) contributions are removed with an exact scalar
correction.  Per-row loss = 15 + ln(exp(diag-15) + S + corr) - diag,
masked by a 0/1 weight vector and row-reduced; the host sums the 8x128
partials and divides by the positive count.
"""

import sys

if "/opt/trn_rl_repo" not in sys.path:
    sys.path.insert(0, "/opt/trn_rl_repo")

from contextlib import ExitStack

import ml_dtypes
import numpy as np

import concourse.bass as bass
import concourse.tile as tile
from concourse import mybir
from concourse.bass_utils import run_bass_kernel_spmd
from concourse.masks import make_identity

TEMPERATURE = 0.07
IGNORE_INDEX = -100
CMAX = 15.0
H = 256
N_CORES = 8

# Stash of the most recent BassKernelResults + shapes (for test harness timing).
LAST_RESULTS = None
LAST_SHAPES = None
TRACE = False


def _legalize_waits(nc: bass.Bass, max_waits: int = 1) -> None:
    """This container's walrus accepts at most one sync-wait per instruction
    (ACT structs especially); Tile can emit several.  Split the excess onto
    same-engine NoOps placed immediately before the instruction."""
    for bb in nc.main_func.blocks:
        new = []
        for ins in bb.instructions:
            si = ins.sync_info
            if si is not None and si.on_wait and len(si.on_wait) > max_waits:
                waits = list(si.on_wait)
                extra, keep = waits[:-max_waits], waits[-max_waits:]
                for i in range(0, len(extra), max_waits):
                    new.append(
                        mybir.InstNoOp(
                            name=nc.get_next_instruction_name(),
                            engine=ins.engine,
                            ins=[],
                            outs=[],
                            sync_info=mybir.SyncInfo(
                                on_wait=extra[i : i + max_waits], on_update=[]
                            ),
                            bass_nofuse=True,
                        )
                    )
                ins.sync_info = mybir.SyncInfo(
                    on_wait=keep, on_update=list(si.on_update or [])
                )
            new.append(ins)
        bb.instructions[:] = new


def _build_program(P1: int, N1: int, legalize: bool = True) -> bass.Bass:
    """One SPMD program: shapes P1 (pos rows) and N1 (neg rows) are uniform
    across cores; per-core data differs via in_maps."""
    PC = P1 // 128
    NC = N1 // 128
    NTILES = N1 // 512
    GROUPS = NC // 4  # 4-chunk transpose groups == 512-wide matmul slabs
    f32 = mybir.dt.float32
    bf16 = mybir.dt.bfloat16
    OP = mybir.AluOpType
    AF = mybir.ActivationFunctionType

    nc = bass.Bass()
    gp = nc.dram_tensor("gp", [P1, H], bf16, kind="ExternalInput")
    ep = nc.dram_tensor("ep", [P1, H], bf16, kind="ExternalInput")
    en = nc.dram_tensor("en", [N1, H], bf16, kind="ExternalInput")
    wv = nc.dram_tensor("wv", [P1], f32, kind="ExternalInput")
    corr = nc.dram_tensor("corr", [1, 1], f32, kind="ExternalInput")
    out = nc.dram_tensor("out", [128, 1], f32, kind="ExternalOutput")

    with tile.TileContext(nc) as tc, ExitStack() as ctx:
        persist = ctx.enter_context(tc.tile_pool(name="persist", bufs=1))
        small = ctx.enter_context(tc.tile_pool(name="small", bufs=1))
        scratch = ctx.enter_context(tc.tile_pool(name="scratch", bufs=3))
        expool = ctx.enter_context(tc.tile_pool(name="expool", bufs=2))
        psum_tp = ctx.enter_context(tc.tile_pool(name="psum_tp", bufs=2, space="PSUM"))
        psum_mm = ctx.enter_context(tc.tile_pool(name="psum_mm", bufs=2, space="PSUM"))

        # ---- constants (gpsimd: otherwise idle) + ACT table preload
        LOG_INV_T = float(-np.log(np.float64(TEMPERATURE)))
        eps_t = small.tile([128, 1], f32)
        nc.gpsimd.memset(eps_t[:], 1e-24)
        blnt_t = small.tile([128, 1], f32)
        nc.gpsimd.memset(blnt_t[:], LOG_INV_T)
        cneg_t = small.tile([128, 1], f32)
        nc.gpsimd.memset(cneg_t[:], -CMAX)
        ident = small.tile([128, 128], bf16)
        make_identity(nc, ident[:])
        # Dummy Ln at t~0 absorbs the ~2.7us ACT table load during the DMAs.
        dummy = small.tile([128, 1], f32)
        nc.scalar.activation(
            out=dummy[:], in_=eps_t[:], func=AF.Ln, bias=eps_t[:, 0:1], scale=1.0
        )

        # ---- loads (bf16), split per 4-chunk piece across the DMA queues
        # (SP + ACT hardware DGE, gpsimd software DGE) so they run in
        # parallel and unblock the pipeline piece by piece.
        # partition i holds rows {c*128+i : c in range(chunks)}
        Gf = persist.tile([128, PC, H], bf16)
        nc.sync.dma_start(out=Gf[:], in_=gp[:].rearrange("(c p) h -> p c h", p=128))
        en_r = en[:].rearrange("(c p) h -> p c h", p=128)
        Np = []
        for g in range(GROUPS):
            t = persist.tile([128, 4, H], bf16, tag=f"np{g}", name=f"np{g}")
            eng = nc.scalar if g % 2 == 1 else nc.sync
            eng.dma_start(out=t[:], in_=en_r[:, g * 4 : (g + 1) * 4, :])
            Np.append(t)
        Ef = persist.tile([128, PC, H], bf16)
        nc.gpsimd.dma_start(out=Ef[:], in_=ep[:].rearrange("(c p) h -> p c h", p=128))
        wt = small.tile([128, PC], f32)
        nc.sync.dma_start(out=wt[:], in_=wv[:].rearrange("(c p) -> p c", p=128))
        corr_t = small.tile([128, 1], f32)
        nc.sync.dma_start(out=corr_t[:], in_=corr[:].to_broadcast([128, 1]))

        # ---- row sums of squares (per 128-row chunk), piece-granular for e
        ssn = []
        for g in range(GROUPS):
            t = small.tile([128, 4], f32, tag=f"ssn{g}", name=f"ssn{g}")
            ssn.append(t)
        ssg = small.tile([128, PC], f32)
        sse = small.tile([128, PC], f32)

        def norm_jobs(xf, c, ss, sc):
            sq = scratch.tile([128, H], bf16, tag="sq")
            nc.vector.scalar_tensor_tensor(
                out=sq[:],
                in0=xf[:, c, :],
                scalar=1.0,
                in1=xf[:, c, :],
                op0=OP.mult,
                op1=OP.mult,
                accum_out=ss[:, sc : sc + 1],
            )

        def scale_of(ss, b):
            # rsqrt as exp(-0.5*ln(ss+eps)): one ACT table set for ln+exp.
            # eps=1e-24 matches the reference's clip(norm, 1e-12).
            nc.scalar.activation(
                out=ss[:], in_=ss[:], func=AF.Ln, bias=eps_t[:, 0:1], scale=1.0
            )
            bias = b if isinstance(b, float) else b[:, 0:1]
            nc.scalar.activation(out=ss[:], in_=ss[:], func=AF.Exp, bias=bias, scale=-0.5)

        # greek norms first (its chain ends at the matmul stationary side),
        # then the e pieces in arrival order
        for c in range(PC):
            norm_jobs(Gf, c, ssg, c)
        scale_of(ssg, blnt_t)  # greek scale carries the 1/T
        for g in range(GROUPS):
            for c in range(4):
                norm_jobs(Np[g], c, ssn[g], c)
            scale_of(ssn[g], 0.0)

        # ---- apply scales -> bf16 matmul operands, on the idle gpsimd
        Gb = persist.tile([128, PC, H], bf16)
        for c in range(PC):
            nc.gpsimd.tensor_scalar_mul(Gb[:, c, :], Gf[:, c, :], ssg[:, c : c + 1])
        Nb = []
        for g in range(GROUPS):
            t = persist.tile([128, 4, H], bf16, tag=f"nb{g}", name=f"nb{g}")
            for c in range(4):
                nc.gpsimd.tensor_scalar_mul(t[:, c, :], Np[g][:, c, :], ssn[g][:, c : c + 1])
            Nb.append(t)

        # ---- transpose to put H on partitions (PE) + copy PSUM->SBUF (DVE)
        GbT = persist.tile([128, 2, P1], bf16)
        for c0 in range(0, PC, 4):
            cn = min(4, PC - c0)
            for hk in range(2):
                pt = psum_tp.tile([128, 512], bf16, tag="pt")
                for j in range(cn):
                    nc.tensor.transpose(
                        pt[:, j * 128 : (j + 1) * 128],
                        Gb[:, c0 + j, hk * 128 : (hk + 1) * 128],
                        ident[:],
                    )
                nc.scalar.copy(
                    out=GbT[:, hk, c0 * 128 : (c0 + cn) * 128], in_=pt[:, : cn * 128]
                )
        NbT = [
            persist.tile([128, 2, 512], bf16, tag=f"nbt{g}", name=f"nbt{g}")
            for g in range(GROUPS)
        ]
        for g in range(GROUPS):
            for hk in range(2):
                pt = psum_tp.tile([128, 512], bf16, tag="pt")
                for j in range(4):
                    nc.tensor.transpose(
                        pt[:, j * 128 : (j + 1) * 128],
                        Nb[g][:, j, hk * 128 : (hk + 1) * 128],
                        ident[:],
                    )
                nc.vector.tensor_copy(out=NbT[g][:, hk, :], in_=pt[:])

        # ---- logits + one fused exp/accumulate pass per 128-row chunk
        # S[p, c] = sum_q exp(logit[c*128+p, q] - CMAX)
        S = small.tile([128, PC], f32)
        for c in range(PC):
            pm = psum_mm.tile([128, N1], f32, tag="pm")
            for nt in range(NTILES):
                for hk in range(2):
                    nc.tensor.matmul(
                        pm[:, nt * 512 : (nt + 1) * 512],
                        GbT[:, hk, c * 128 : (c + 1) * 128],
                        NbT[nt][:, hk, :],
                        start=(hk == 0),
                        stop=(hk == 1),
                    )
            ex = expool.tile([128, N1], f32, tag="ex")
            nc.scalar.activation(
                out=ex[:],
                in_=pm[:],
                func=AF.Exp,
                bias=cneg_t[:, 0:1],
                scale=1.0,
                accum_out=S[:, c : c + 1],
            )

        # ---- diag[p] = raw greek.english dot, scaled by both row norms
        for c in range(PC):
            norm_jobs(Ef, c, sse, c)
        scale_of(sse, 0.0)
        diag = small.tile([128, PC], f32)
        for c in range(PC):
            dsq = scratch.tile([128, H], bf16, tag="dsq")
            nc.vector.scalar_tensor_tensor(
                out=dsq[:],
                in0=Gf[:, c, :],
                scalar=1.0,
                in1=Ef[:, c, :],
                op0=OP.mult,
                op1=OP.mult,
                accum_out=diag[:, c : c + 1],
            )
        nc.vector.tensor_mul(diag[:], diag[:], ssg[:])
        nc.vector.tensor_mul(diag[:], diag[:], sse[:])

        # ---- per-row loss and masked partial sum
        ed = small.tile([128, PC], f32)
        nc.scalar.activation(
            out=ed[:], in_=diag[:], func=AF.Exp, bias=cneg_t[:, 0:1], scale=1.0
        )
        t2 = small.tile([128, PC], f32)
        nc.vector.scalar_tensor_tensor(
            out=t2[:],
            in0=S[:],
            scalar=corr_t[:, 0:1],
            in1=ed[:],
            op0=OP.add,
            op1=OP.add,
        )
        nc.scalar.activation(out=t2[:], in_=t2[:], func=AF.Ln)
        # loss = (ln(...) + CMAX) - diag
        loss = small.tile([128, PC], f32)
        nc.vector.scalar_tensor_tensor(
            out=loss[:],
            in0=t2[:],
            scalar=CMAX,
            in1=diag[:],
            op0=OP.add,
            op1=OP.subtract,
        )
        lm = small.tile([128, PC], f32)
        part = small.tile([128, 1], f32)
        nc.vector.scalar_tensor_tensor(
            out=lm[:],
            in0=loss[:],
            scalar=1.0,
            in1=wt[:],
            op0=OP.mult,
            op1=OP.mult,
            accum_out=part[:],
        )
        nc.sync.dma_start(out=out[:], in_=part[:])
    if legalize:
        _legalize_waits(nc, max_waits=1)
    return nc


def _pad_rows(x: np.ndarray, n: int) -> np.ndarray:
    outp = np.zeros((n,) + x.shape[1:], dtype=x.dtype)
    outp[: x.shape[0]] = x
    return outp


def kernel(greek_embeds, english_embeds, labels):
    global LAST_RESULTS
    g = np.ascontiguousarray(np.asarray(greek_embeds, dtype=np.float32))
    e = np.ascontiguousarray(np.asarray(english_embeds, dtype=np.float32))
    lab = np.asarray(labels)
    B, P, Hh = g.shape
    assert Hh == H and B * 2 == N_CORES

    valid = lab != IGNORE_INDEX
    pos = valid & (lab == 1)
    neg = valid & (lab != 1)
    ok = (valid.sum(-1) >= 2) & pos.any(-1) & neg.any(-1)

    count = int(pos[ok].sum()) if ok.any() else 0
    if count == 0:
        return np.float32(0.0)

    pos_idx = [np.nonzero(pos[b])[0] if ok[b] else np.zeros(0, np.int64) for b in range(B)]
    neg_idx = [np.nonzero(neg[b])[0] if ok[b] else np.zeros(0, np.int64) for b in range(B)]
    halves = [np.array_split(pi, 2) for pi in pos_idx]

    np_max = max(len(halves[b][h]) for b in range(B) for h in range(2))
    nn_max = max(len(ni) for ni in neg_idx)
    P1 = max(128, ((np_max + 127) // 128) * 128)
    N1 = max(512, ((nn_max + 511) // 512) * 512)

    E15 = np.float32(np.exp(np.float32(-CMAX)))
    in_maps = []
    for core in range(N_CORES):
        b, hf = core // 2, core % 2
        p_idx = halves[b][hf]
        n_idx = neg_idx[b]
        w = np.zeros(P1, np.float32)
        w[: len(p_idx)] = 1.0
        in_maps.append(
            {
                "gp": _pad_rows(g[b][p_idx].astype(ml_dtypes.bfloat16), P1),
                "ep": _pad_rows(e[b][p_idx].astype(ml_dtypes.bfloat16), P1),
                "en": _pad_rows(e[b][n_idx].astype(ml_dtypes.bfloat16), N1),
                "wv": w,
                "corr": np.array([[-(N1 - len(n_idx)) * float(E15)]], np.float32),
            }
        )

    global LAST_SHAPES
    LAST_SHAPES = (P1, N1, dict(in_maps[0]))
    nc = _build_program(P1, N1)
    res = run_bass_kernel_spmd(nc, in_maps, list(range(N_CORES)), trace=TRACE)
    LAST_RESULTS = res
    total = sum(float(r["out"].sum()) for r in res.results)
    return np.float32(total / count)



# revision 2
# speedup vs baseline: 1.0577x; 1.0577x over previous
"""Contrastive loss kernel for Trainium2 (8 NeuronCores, Bass/Tile).

Strategy
--------
Only rows with label==1 (pos) contribute losses, and only columns with
label==0 (neg) plus the diagonal enter each row's logsumexp.  The host
computes the tiny index sets from `labels`, then each of the 8 cores
(2 per batch) receives its half of the batch's positive rows and all of
the batch's negative english rows, padded to uniform shapes (P1, N1).

Device pipeline (all reference FLOPs on device; host only gathers /
pads / transposes / casts):
  - g arrives twice: raw fp8e4 (x8) transposed [h, ktile, p] as the
    DoubleRow matmul stationary, and bf16 rows for norms + diag dots.
    g's normalization is folded into the exp's per-partition scale
    SG_p = 1/(64*T*|g_p|), so g is never scaled on device.
  - e-neg rows arrive bf16; per 128-row chunk: sum-of-squares (DVE,
    accum), rsqrt via Ln/Exp (ACT), diag(s_q) built from the identity
    (tensor_scalar), then ONE regular matmul per 128-col half computes
    transpose-and-scale fused: out[h, q] = e[q, h] * s_q.  The PSUM
    result is copied+cast to fp8e4 SBUF on a rotating engine.
  - Main matmuls run in fp8e4 DoubleRow perf mode: one matmul per
    512-col slab contracts all H=256 (2 k-tiles) at 0.5 cycles/row.
  - exp(logit*SG_p - 15) runs in-place on the PSUM tile with accum_out
    giving the per-row negative sums S.  A fixed max constant 15 >
    1/0.07 bounds the logits so the logsumexp stays exact in f32.
    Zero-padded columns give exactly exp(-15), removed by a scalar
    correction.  The first row-chunk's exp is split by slab so ACT can
    start before the full eT is ready.
  - diag: bf16 row dots (DVE, accum) scaled by both rsqrt norms; the
    per-row loss is ln(exp(diag-15)+S+corr)+15-diag, masked and
    row-reduced; host sums the 8x128 partials and divides by count.
"""

import sys

if "/opt/trn_rl_repo" not in sys.path:
    sys.path.insert(0, "/opt/trn_rl_repo")

from contextlib import ExitStack

import ml_dtypes
import numpy as np

import concourse.bass as bass
import concourse.tile as tile
from concourse import mybir
from concourse.bass_utils import run_bass_kernel_spmd
from concourse.masks import make_identity

TEMPERATURE = 0.07
IGNORE_INDEX = -100
CMAX = 15.0
H = 256
N_CORES = 8
FP8_SCALE = 8.0  # keeps fp8e4 operands out of the subnormal range

# Stash of the most recent BassKernelResults + shapes (for test harness timing).
LAST_RESULTS = None
LAST_SHAPES = None
TRACE = False


def _legalize_waits(nc: bass.Bass, max_waits: int = 1) -> None:
    """This container's walrus accepts at most one sync-wait per instruction
    (ACT structs especially); Tile can emit several.  Split the excess onto
    same-engine NoOps placed immediately before the instruction."""
    for bb in nc.main_func.blocks:
        new = []
        for ins in bb.instructions:
            si = ins.sync_info
            if si is not None and si.on_wait and len(si.on_wait) > max_waits:
                waits = list(si.on_wait)
                extra, keep = waits[:-max_waits], waits[-max_waits:]
                for i in range(0, len(extra), max_waits):
                    new.append(
                        mybir.InstNoOp(
                            name=nc.get_next_instruction_name(),
                            engine=ins.engine,
                            ins=[],
                            outs=[],
                            sync_info=mybir.SyncInfo(
                                on_wait=extra[i : i + max_waits], on_update=[]
                            ),
                            bass_nofuse=True,
                        )
                    )
                ins.sync_info = mybir.SyncInfo(
                    on_wait=keep, on_update=list(si.on_update or [])
                )
            new.append(ins)
        bb.instructions[:] = new


def _build_program(P1: int, N1: int, legalize: bool = True) -> bass.Bass:
    """One SPMD program: shapes P1 (pos rows) and N1 (neg cols) are uniform
    across cores; per-core data differs via in_maps."""
    PC = P1 // 128
    NC = N1 // 128
    NG = (NC + 3) // 4  # e-row DMA/rsqrt groups of up to 4 chunks
    slabs = [(s, min(s + 512, N1)) for s in range(0, N1, 512)]
    f32 = mybir.dt.float32
    bf16 = mybir.dt.bfloat16
    fp8 = mybir.dt.float8e4
    OP = mybir.AluOpType
    AF = mybir.ActivationFunctionType
    DR = mybir.MatmulPerfMode.DoubleRow

    nc = bass.Bass()
    g8t = nc.dram_tensor("g8t", [128, 2, P1], fp8, kind="ExternalInput")
    gb = nc.dram_tensor("gb", [128, PC, H], bf16, kind="ExternalInput")
    ep = nc.dram_tensor("ep", [128, PC, H], bf16, kind="ExternalInput")
    en = nc.dram_tensor("en", [128, NC, H], bf16, kind="ExternalInput")
    wv = nc.dram_tensor("wv", [128, PC], f32, kind="ExternalInput")
    corr = nc.dram_tensor("corr", [1, 1], f32, kind="ExternalInput")
    out = nc.dram_tensor("out", [128, 1], f32, kind="ExternalOutput")

    with tile.TileContext(nc) as tc, ExitStack() as ctx:
        persist = ctx.enter_context(tc.tile_pool(name="persist", bufs=1))
        small = ctx.enter_context(tc.tile_pool(name="small", bufs=1))
        scratch = ctx.enter_context(tc.tile_pool(name="scratch", bufs=3))
        dgpool = ctx.enter_context(tc.tile_pool(name="dgpool", bufs=3))
        psum_tp = ctx.enter_context(tc.tile_pool(name="psum_tp", bufs=2, space="PSUM"))
        psum_mm = ctx.enter_context(tc.tile_pool(name="psum_mm", bufs=2, space="PSUM"))

        # ---- constants (gpsimd: otherwise idle early) + ACT table preload
        eps_t = small.tile([128, 1], f32)
        nc.gpsimd.memset(eps_t[:], 1e-24)
        # e-side: s_q = 8*rsqrt(ssn)         -> Exp bias ln(8)
        ln8_t = small.tile([128, 1], f32)
        nc.gpsimd.memset(ln8_t[:], float(np.log(FP8_SCALE)))
        # g-side: SG_p = rsqrt(ssg)/(64*T)   -> Exp bias -ln(64*T)
        mg_t = small.tile([128, 1], f32)
        nc.gpsimd.memset(mg_t[:], float(-np.log(FP8_SCALE * FP8_SCALE * TEMPERATURE)))
        # ep-side: rse_p = 64*T*rsqrt(sse)... diag = dg*SG*rse with
        # rse = 64*rsqrt(sse) so that dg*SG*rse = dg/(|g||e|T). bias ln(64).
        ln64_t = small.tile([128, 1], f32)
        nc.gpsimd.memset(ln64_t[:], float(np.log(FP8_SCALE * FP8_SCALE)))
        cneg_t = small.tile([128, 1], f32)
        nc.gpsimd.memset(cneg_t[:], -CMAX)
        ident = small.tile([128, 128], bf16)
        make_identity(nc, ident[:])
        # Dummy Ln at t~0 absorbs the ACT table load during the DMAs.
        dummy = small.tile([128, 1], f32)
        nc.scalar.activation(
            out=dummy[:], in_=eps_t[:], func=AF.Ln, bias=eps_t[:, 0:1], scale=1.0
        )

        # ---- loads: e-neg groups first (they gate everything), g8t early on
        # the scalar queue (gates mm0), gb early on sync (gates SG/exp0).
        En = []
        for g in range(NG):
            k = min(4, NC - 4 * g)
            t = persist.tile([128, k, H], bf16, tag=f"en{g}", name=f"en{g}")
            eng = nc.scalar if g == 1 else nc.sync
            eng.dma_start(out=t[:], in_=en[:, 4 * g : 4 * g + k, :])
            En.append(t)
        G8 = persist.tile([128, 2, P1], fp8)
        nc.scalar.dma_start(out=G8[:], in_=g8t[:])
        Gb = persist.tile([128, PC, H], bf16)
        nc.sync.dma_start(out=Gb[:], in_=gb[:])
        Ef = persist.tile([128, PC, H], bf16)
        nc.scalar.dma_start(out=Ef[:], in_=ep[:])
        wt = small.tile([128, PC], f32)
        nc.sync.dma_start(out=wt[:], in_=wv[:])
        corr_t = small.tile([128, 1], f32)
        nc.sync.dma_start(out=corr_t[:], in_=corr[:].to_broadcast([128, 1]))

        ssn = small.tile([128, NC], f32)
        ssg = small.tile([128, PC], f32)
        sse = small.tile([128, PC], f32)
        eT8 = persist.tile([128, 2, N1], fp8)

        def square_acc(eng, src, acc):
            sq = scratch.tile([128, H], bf16, tag="sq")
            eng.scalar_tensor_tensor(
                out=sq[:],
                in0=src,
                scalar=1.0,
                in1=src,
                op0=OP.mult,
                op1=OP.mult,
                accum_out=acc,
            )

        def rsqrt_of(ss, bias):
            # rsqrt as exp(-0.5*ln(ss+eps)); eps matches clip(norm, 1e-12).
            nc.scalar.activation(out=ss, in_=ss, func=AF.Ln, bias=eps_t[:, 0:1], scale=1.0)
            nc.scalar.activation(out=ss, in_=ss, func=AF.Exp, bias=bias[:, 0:1], scale=-0.5)

        # ---- e-prep pipeline; g-squares interleave after the first group so
        # SG is ready before exp0a.  Engine rotation chosen to balance load.
        diag_eng = [nc.vector, nc.gpsimd]  # alternating
        copy_eng = [nc.gpsimd, nc.vector, nc.scalar]

        def e_chunk(c):
            # diag(s) for this chunk
            dg = dgpool.tile([128, 128], bf16, tag="dg")
            diag_eng[c % 2].tensor_scalar_mul(dg[:], ident[:], ssn[:, c : c + 1])
            # transpose-and-scale via one matmul per h-half
            pt = psum_tp.tile([128, 2, 128], f32, tag="pt")
            g = c // 4
            for hk in range(2):
                nc.tensor.matmul(
                    pt[:, hk, :],
                    En[g][:, c - 4 * g, hk * 128 : (hk + 1) * 128],
                    dg[:],
                    start=True,
                    stop=True,
                )
            # copy+cast PSUM f32 -> SBUF fp8 (both h-halves in one op)
            eng = copy_eng[c % 3]
            if eng is nc.scalar:
                nc.scalar.copy(out=eT8[:, :, c * 128 : (c + 1) * 128], in_=pt[:])
            else:
                eng.tensor_copy(out=eT8[:, :, c * 128 : (c + 1) * 128], in_=pt[:])

        # squares group 0 + its rsqrt
        for c in range(4):
            square_acc(nc.vector, En[0][:, c, :], ssn[:, c : c + 1])
        rsqrt_of(ssn[:, 0:4], ln8_t)
        for c in range(4):
            e_chunk(c)
        # g-squares now (DVE) so SG is ready for exp0a
        for c in range(PC):
            square_acc(nc.vector, Gb[:, c, :], ssg[:, c : c + 1])
        rsqrt_of(ssg[:], mg_t)  # ssg now holds SG = rsqrt/(64T)
        # remaining e groups
        for g in range(1, NG):
            k = min(4, NC - 4 * g)
            for j in range(k):
                square_acc(nc.vector, En[g][:, j, :], ssn[:, 4 * g + j : 4 * g + j + 1])
            rsqrt_of(ssn[:, 4 * g : 4 * g + k], ln8_t)
            for j in range(k):
                e_chunk(4 * g + j)

        # ---- main loop: fp8 DoubleRow matmuls + exp/accumulate
        # S layout: [0]=chunk0 slab0, [1]=chunk0 slabs1.., [c+1]=chunk c.
        S = small.tile([128, PC + 1], f32)
        for c in range(PC):
            pm = psum_mm.tile([128, 3 * 512], f32, tag="pm")
            for s0, s1 in slabs:
                nc.tensor.matmul(
                    pm[:, s0:s1],
                    G8[:, :, c * 128 : (c + 1) * 128],
                    eT8[:, :, s0:s1],
                    start=True,
                    stop=True,
                    perf_mode=DR,
                )
            if c == 0:
                # split so ACT starts before the last e-chunks are ready
                nc.scalar.activation(
                    out=pm[:, 0:512],
                    in_=pm[:, 0:512],
                    func=AF.Exp,
                    bias=cneg_t[:, 0:1],
                    scale=ssg[:, c : c + 1],
                    accum_out=S[:, 0:1],
                )
                nc.scalar.activation(
                    out=pm[:, 512:N1],
                    in_=pm[:, 512:N1],
                    func=AF.Exp,
                    bias=cneg_t[:, 0:1],
                    scale=ssg[:, c : c + 1],
                    accum_out=S[:, 1:2],
                )
            else:
                nc.scalar.activation(
                    out=pm[:, 0:N1],
                    in_=pm[:, 0:N1],
                    func=AF.Exp,
                    bias=cneg_t[:, 0:1],
                    scale=ssg[:, c : c + 1],
                    accum_out=S[:, c + 1 : c + 2],
                )
            if c == 0:
                # fold the slab0 partial into S[1] (hidden under exp1)
                nc.vector.tensor_tensor(
                    out=S[:, 1:2], in0=S[:, 0:1], in1=S[:, 1:2], op=OP.add
                )

        # ---- diag path (hidden under the exp phase on DVE)
        for c in range(PC):
            square_acc(nc.vector, Ef[:, c, :], sse[:, c : c + 1])
        rsqrt_of(sse[:], ln64_t)  # sse now holds 64*rsqrt(sse)
        dgots = small.tile([128, PC], f32)
        for c in range(PC):
            dsq = scratch.tile([128, H], bf16, tag="dsq")
            nc.vector.scalar_tensor_tensor(
                out=dsq[:],
                in0=Gb[:, c, :],
                scalar=1.0,
                in1=Ef[:, c, :],
                op0=OP.mult,
                op1=OP.mult,
                accum_out=dgots[:, c : c + 1],
            )
        diag = small.tile([128, PC], f32)
        nc.vector.tensor_mul(diag[:], dgots[:], ssg[:])
        nc.vector.tensor_mul(diag[:], diag[:], sse[:])
        ed = small.tile([128, PC], f32)
        nc.scalar.activation(
            out=ed[:], in_=diag[:], func=AF.Exp, bias=cneg_t[:, 0:1], scale=1.0
        )

        # ---- per-row loss and masked partial sum
        t2 = small.tile([128, PC], f32)
        nc.vector.scalar_tensor_tensor(
            out=t2[:],
            in0=S[:, 1 : PC + 1],
            scalar=corr_t[:, 0:1],
            in1=ed[:],
            op0=OP.add,
            op1=OP.add,
        )
        nc.scalar.activation(out=t2[:], in_=t2[:], func=AF.Ln, bias=eps_t[:, 0:1], scale=1.0)
        loss = small.tile([128, PC], f32)
        nc.vector.scalar_tensor_tensor(
            out=loss[:],
            in0=t2[:],
            scalar=CMAX,
            in1=diag[:],
            op0=OP.add,
            op1=OP.subtract,
        )
        lm = small.tile([128, PC], f32)
        part = small.tile([128, 1], f32)
        nc.vector.scalar_tensor_tensor(
            out=lm[:],
            in0=loss[:],
            scalar=1.0,
            in1=wt[:],
            op0=OP.mult,
            op1=OP.mult,
            accum_out=part[:],
        )
        nc.sync.dma_start(out=out[:], in_=part[:])
    if legalize:
        _legalize_waits(nc, max_waits=1)
    return nc


def _pad_rows(x: np.ndarray, n: int) -> np.ndarray:
    outp = np.zeros((n,) + x.shape[1:], dtype=x.dtype)
    outp[: x.shape[0]] = x
    return outp


def kernel(greek_embeds, english_embeds, labels):
    global LAST_RESULTS, LAST_SHAPES
    g = np.ascontiguousarray(np.asarray(greek_embeds, dtype=np.float32))
    e = np.ascontiguousarray(np.asarray(english_embeds, dtype=np.float32))
    lab = np.asarray(labels)
    B, P, Hh = g.shape
    assert Hh == H and B * 2 == N_CORES

    valid = lab != IGNORE_INDEX
    pos = valid & (lab == 1)
    neg = valid & (lab != 1)
    ok = (valid.sum(-1) >= 2) & pos.any(-1) & neg.any(-1)

    count = int(pos[ok].sum()) if ok.any() else 0
    if count == 0:
        return np.float32(0.0)

    pos_idx = [np.nonzero(pos[b])[0] if ok[b] else np.zeros(0, np.int64) for b in range(B)]
    neg_idx = [np.nonzero(neg[b])[0] if ok[b] else np.zeros(0, np.int64) for b in range(B)]
    halves = [np.array_split(pi, 2) for pi in pos_idx]

    np_max = max(len(halves[b][h]) for b in range(B) for h in range(2))
    nn_max = max(len(ni) for ni in neg_idx)
    P1 = max(128, ((np_max + 127) // 128) * 128)
    N1 = max(512, ((nn_max + 127) // 128) * 128)
    PC, NC = P1 // 128, N1 // 128

    E15 = np.float32(np.exp(np.float32(-CMAX)))
    bf16 = ml_dtypes.bfloat16
    fp8 = ml_dtypes.float8_e4m3
    in_maps = []
    for core in range(N_CORES):
        b, hf = core // 2, core % 2
        p_idx = halves[b][hf]
        n_idx = neg_idx[b]
        gr = _pad_rows(g[b][p_idx], P1)                      # [P1, 256] f32
        er = _pad_rows(e[b][n_idx], N1)                      # [N1, 256] f32
        epr = _pad_rows(e[b][p_idx], P1)
        w = np.zeros((128, PC), np.float32)
        npos = len(p_idx)
        for c in range(PC):
            base = c * 128
            w[: max(0, min(128, npos - base)), c] = 1.0
        in_maps.append(
            {
                # stationary: raw g x8, [h%128, ktile, row]
                "g8t": np.ascontiguousarray(
                    (gr * FP8_SCALE).reshape(P1, 2, 128).transpose(2, 1, 0)
                ).astype(fp8),
                # row layouts: row c*128+p -> [p, c, :]
                "gb": np.ascontiguousarray(
                    gr.astype(bf16).reshape(PC, 128, H).transpose(1, 0, 2)
                ),
                "ep": np.ascontiguousarray(
                    epr.astype(bf16).reshape(PC, 128, H).transpose(1, 0, 2)
                ),
                "en": np.ascontiguousarray(
                    er.astype(bf16).reshape(NC, 128, H).transpose(1, 0, 2)
                ),
                "wv": w,
                "corr": np.array([[-(N1 - len(n_idx)) * float(E15)]], np.float32),
            }
        )

    LAST_SHAPES = (P1, N1, dict(in_maps[0]))
    nc = _build_program(P1, N1)
    res = run_bass_kernel_spmd(nc, in_maps, list(range(N_CORES)), trace=TRACE)
    LAST_RESULTS = res
    total = sum(float(r["out"].sum()) for r in res.results)
    return np.float32(total / count)


# revision 3
# speedup vs baseline: 1.1694x; 1.1056x over previous
"""Contrastive loss kernel for Trainium2 (8 NeuronCores, Bass/Tile).

Strategy
--------
Only rows with label==1 (pos) contribute losses, and only columns with
label==0 (neg) plus the diagonal enter each row's logsumexp.  The host
computes the tiny index sets from `labels`, then each of the 8 cores
(2 per batch) receives its half of the batch's positive rows and all of
the batch's negative english rows, padded to uniform shapes (P1, N1).

Device pipeline (all reference FLOPs on device; host only gathers /
pads / transposes / casts):
  - g arrives twice: raw fp8e4 (x8) transposed [h%128, ktile, row] as
    the DoubleRow matmul stationary, and bf16 rows for norms + diag
    dots.  g's normalization folds into the exp's per-partition scale
    SG_p = 1/(64*T*|g_p|), so g is never scaled on device.
  - e-neg rows arrive bf16; per 128-row chunk: sum-of-squares (DVE or
    GPSIMD, accum), rsqrt via Ln/Exp (ACT), diag(s_q) built by an
    affine_select from a broadcast (GPSIMD), then one regular matmul
    per h-half computes transpose-and-scale fused:
    out[h, q] = e[q, h] * s_q.  PSUM -> SBUF fp8 copy on DVE/GPSIMD.
  - Main matmuls run in fp8e4 DoubleRow perf mode: one matmul per
    512-col slab contracts all H=256 (2 k-tiles) at 0.5 cycles/row.
  - exp(logit*SG_p - 15) runs in-place on the PSUM tile with accum_out
    giving per-row negative sums S.  Chunk 0 is exp'd per slab so ACT
    starts before the full eT is ready.  Padded columns give exactly
    exp(-15), removed by a scalar correction.
  - diag: bf16 row dots (DVE, accum) scaled by both rsqrt norms; the
    per-row loss is ln(exp(diag-15)+S+corr)+15-diag, masked and
    row-reduced; host sums the 8x128 partials and divides by count.
"""

import sys

if "/opt/trn_rl_repo" not in sys.path:
    sys.path.insert(0, "/opt/trn_rl_repo")

from contextlib import ExitStack

import ml_dtypes
import numpy as np

import concourse.bass as bass
import concourse.tile as tile
from concourse import mybir
from concourse.bass_utils import run_bass_kernel_spmd
from concourse.masks import make_identity

TEMPERATURE = 0.07
IGNORE_INDEX = -100
CMAX = 15.0
H = 256
N_CORES = 8
FP8_SCALE = 8.0  # keeps fp8e4 operands away from the subnormal range

LAST_RESULTS = None
LAST_SHAPES = None
TRACE = False


def _legalize_waits(nc: bass.Bass, max_waits: int = 1) -> None:
    """This container's walrus accepts at most one sync-wait per instruction
    (ACT structs especially); Tile can emit several.  Split the excess onto
    same-engine NoOps placed immediately before the instruction."""
    for bb in nc.main_func.blocks:
        new = []
        for ins in bb.instructions:
            si = ins.sync_info
            if si is not None and si.on_wait and len(si.on_wait) > max_waits:
                waits = list(si.on_wait)
                extra, keep = waits[:-max_waits], waits[-max_waits:]
                for i in range(0, len(extra), max_waits):
                    new.append(
                        mybir.InstNoOp(
                            name=nc.get_next_instruction_name(),
                            engine=ins.engine,
                            ins=[],
                            outs=[],
                            sync_info=mybir.SyncInfo(
                                on_wait=extra[i : i + max_waits], on_update=[]
                            ),
                            bass_nofuse=True,
                        )
                    )
                ins.sync_info = mybir.SyncInfo(
                    on_wait=keep, on_update=list(si.on_update or [])
                )
            new.append(ins)
        bb.instructions[:] = new


def _build_program(P1: int, N1: int, legalize: bool = True) -> bass.Bass:
    PC = P1 // 128
    NC = N1 // 128
    slabs = [(s, min(s + 512, N1)) for s in range(0, N1, 512)]
    NS = len(slabs)
    f32 = mybir.dt.float32
    bf16 = mybir.dt.bfloat16
    fp8 = mybir.dt.float8e4
    OP = mybir.AluOpType
    AF = mybir.ActivationFunctionType
    DR = mybir.MatmulPerfMode.DoubleRow

    nc = bass.Bass()
    g8t = nc.dram_tensor("g8t", [128, 2, P1], fp8, kind="ExternalInput")
    gb = nc.dram_tensor("gb", [128, PC, H], bf16, kind="ExternalInput")
    ep = nc.dram_tensor("ep", [128, PC, H], bf16, kind="ExternalInput")
    en = nc.dram_tensor("en", [128, NC, H], bf16, kind="ExternalInput")
    wv = nc.dram_tensor("wv", [128, PC], f32, kind="ExternalInput")
    corr = nc.dram_tensor("corr", [1, 1], f32, kind="ExternalInput")
    out = nc.dram_tensor("out", [128, 1], f32, kind="ExternalOutput")

    # e-chunk groups: 0-3 (DVE squares), 4-7, 8-10 (GPSIMD squares)
    g_of = lambda c: min(c // 4, 2)

    with tile.TileContext(nc) as tc, ExitStack() as ctx:
        persist = ctx.enter_context(tc.tile_pool(name="persist", bufs=1))
        small = ctx.enter_context(tc.tile_pool(name="small", bufs=1))
        scratch = ctx.enter_context(tc.tile_pool(name="scratch", bufs=3))
        dgpool = ctx.enter_context(tc.tile_pool(name="dgpool", bufs=3))
        psum_tp = ctx.enter_context(tc.tile_pool(name="psum_tp", bufs=2, space="PSUM"))
        psum_mm = ctx.enter_context(tc.tile_pool(name="psum_mm", bufs=2, space="PSUM"))

        # ---- constants + ACT table preload
        eps_t = small.tile([128, 1], f32)
        nc.gpsimd.memset(eps_t[:], 1e-24)
        ln8_t = small.tile([128, 1], f32)
        nc.gpsimd.memset(ln8_t[:], float(np.log(FP8_SCALE)))
        mg_t = small.tile([128, 1], f32)
        nc.gpsimd.memset(mg_t[:], float(-np.log(FP8_SCALE * FP8_SCALE * TEMPERATURE)))
        ln64_t = small.tile([128, 1], f32)
        nc.gpsimd.memset(ln64_t[:], float(np.log(FP8_SCALE * FP8_SCALE)))
        cneg_t = small.tile([128, 1], f32)
        nc.gpsimd.memset(cneg_t[:], -CMAX)

        # scalar queue: ONLY en group 1 (frees ACT seq quickly), then dummy Ln
        # absorbs the ~1.3us ACT table load while DVE/GPSIMD square.
        En = []
        n_groups = [list(range(0, min(4, NC))), list(range(4, min(8, NC))), list(range(8, NC))]
        n_groups = [g for g in n_groups if g]
        for gi, chunks in enumerate(n_groups):
            t = persist.tile([128, len(chunks), H], bf16, tag=f"en{gi}", name=f"en{gi}")
            En.append(t)
        if len(n_groups) > 1:
            nc.scalar.dma_start(out=En[1][:], in_=en[:, n_groups[1][0] : n_groups[1][-1] + 1, :])
        dummy = small.tile([128, 1], f32)
        nc.scalar.activation(
            out=dummy[:], in_=eps_t[:], func=AF.Ln, bias=eps_t[:, 0:1], scale=1.0
        )
        # sync queue, priority order
        nc.sync.dma_start(out=En[0][:], in_=en[:, n_groups[0][0] : n_groups[0][-1] + 1, :])
        Gb = persist.tile([128, PC, H], bf16)
        nc.sync.dma_start(out=Gb[:], in_=gb[:])
        if len(n_groups) > 2:
            nc.sync.dma_start(out=En[2][:], in_=en[:, n_groups[2][0] : n_groups[2][-1] + 1, :])
        G8 = persist.tile([128, 2, P1], fp8)
        nc.sync.dma_start(out=G8[:], in_=g8t[:])
        Ef = persist.tile([128, PC, H], bf16)
        nc.sync.dma_start(out=Ef[:], in_=ep[:])
        wt = small.tile([128, PC], f32)
        nc.sync.dma_start(out=wt[:], in_=wv[:])
        corr_t = small.tile([128, 1], f32)
        nc.sync.dma_start(out=corr_t[:], in_=corr[:].to_broadcast([128, 1]))

        ident = small.tile([128, 128], bf16)
        make_identity(nc, ident[:])

        ssn = small.tile([128, NC], f32)
        ssg = small.tile([128, PC], f32)
        sse = small.tile([128, PC], f32)
        eT8 = persist.tile([128, 2, N1], fp8)

        def square_acc(eng, src, acc):
            sq = scratch.tile([128, H], bf16, tag="sq")
            eng.scalar_tensor_tensor(
                out=sq[:], in0=src, scalar=1.0, in1=src,
                op0=OP.mult, op1=OP.mult, accum_out=acc,
            )

        def rsqrt_of(ss, bias):
            nc.scalar.activation(out=ss, in_=ss, func=AF.Ln, bias=eps_t[:, 0:1], scale=1.0)
            nc.scalar.activation(out=ss, in_=ss, func=AF.Exp, bias=bias[:, 0:1], scale=-0.5)

        def en_src(c):
            gi = g_of(c)
            return En[gi][:, c - n_groups[gi][0], :]

        def diag_build(c):
            dg = dgpool.tile([128, 128], bf16, tag="dg")
            nc.gpsimd.affine_select(
                out=dg[:],
                in_=ssn[:, c : c + 1].to_broadcast([128, 128]),
                compare_op=OP.is_equal,
                fill=0.0,
                base=0,
                pattern=[[-1, 128]],
                channel_multiplier=1,
            )
            return dg

        def transp(c, dg):
            pt = psum_tp.tile([128, 2, 128], f32, tag="pt")
            for hk in range(2):
                nc.tensor.matmul(
                    pt[:, hk, :],
                    en_src(c)[:, hk * 128 : (hk + 1) * 128],
                    dg[:],
                    start=True,
                    stop=True,
                )
            return pt

        def copy_out(eng, c, pt):
            dst = eT8[:, :, c * 128 : (c + 1) * 128]
            if eng is nc.scalar:
                nc.scalar.copy(out=dst, in_=pt[:])
            else:
                eng.tensor_copy(out=dst, in_=pt[:])

        def main_mm(c, si):
            s0, s1 = slabs[si]
            nc.tensor.matmul(
                pm_tiles[c][:, s0:s1],
                G8[:, :, c * 128 : (c + 1) * 128],
                eT8[:, :, s0:s1],
                start=True,
                stop=True,
                perf_mode=DR,
            )

        # S layout: [0..NS-1] = chunk-0 slab partials; [NS-1+c] = chunk c>=1.
        S = small.tile([128, NS + PC - 1], f32)

        def exp_acc(c, col_lo, col_hi, s_col):
            nc.scalar.activation(
                out=pm_tiles[c][:, col_lo:col_hi],
                in_=pm_tiles[c][:, col_lo:col_hi],
                func=AF.Exp,
                bias=cneg_t[:, 0:1],
                scale=ssg[:, c : c + 1],
                accum_out=S[:, s_col : s_col + 1],
            )

        # ---------------- emission (per-engine queues are in-order) --------
        # DVE: e-sq 0-3
        for c in n_groups[0]:
            square_acc(nc.vector, en_src(c), ssn[:, c : c + 1])
        # GPSIMD: e-sq 4-10 (after its memsets/identity)
        for gi in range(1, len(n_groups)):
            for c in n_groups[gi]:
                square_acc(nc.gpsimd, en_src(c), ssn[:, c : c + 1])
        # ACT: rsqrt group 0, then g chunk 0
        rsqrt_of(ssn[:, 0 : len(n_groups[0])], ln8_t)
        # DVE: g squares (all chunks; chunk0 first so SG0 is early)
        for c in range(PC):
            square_acc(nc.vector, Gb[:, c, :], ssg[:, c : c + 1])
        rsqrt_of(ssg[:, 0:1], mg_t)

        # GPSIMD: diags 0-3; transposes 0-3 (PE); copies 0-3 (DVE+GPSIMD)
        pm_tiles = {}
        for c in n_groups[0]:
            dg = diag_build(c)
            pt = transp(c, dg)
            copy_out(nc.vector if c % 2 == 0 else nc.gpsimd, c, pt)
        # PE: main slab-0 matmuls for chunks 0,1
        for c in range(min(2, PC)):
            pm_tiles[c] = psum_mm.tile([128, 512 * ((N1 * 4 + 2047) // 2048)], f32, tag="pm", name=f"pm{c}")
            main_mm(c, 0)
        # ACT: exp chunk0 slab0
        exp_acc(0, 0, slabs[0][1], 0)

        # rsqrt for e-groups 1,2 and g 1..PC
        if len(n_groups) > 1:
            rsqrt_of(ssn[:, n_groups[1][0] : n_groups[1][-1] + 1], ln8_t)
        if PC > 1:
            rsqrt_of(ssg[:, 1:PC], mg_t)
        if len(n_groups) > 2:
            rsqrt_of(ssn[:, n_groups[2][0] : n_groups[2][-1] + 1], ln8_t)

        # e-pipe for groups 1,2 + per-slab main matmuls and chunk-0 exps
        for gi in range(1, len(n_groups)):
            for c in n_groups[gi]:
                dg = diag_build(c)
                pt = transp(c, dg)
                copy_out(nc.vector if c % 2 == 0 else nc.gpsimd, c, pt)
            # slab gi of chunks 0,1 now has all its e-chunks
            for c in range(min(2, PC)):
                main_mm(c, gi)
            exp_acc(0, slabs[gi][0], slabs[gi][1], gi)
        if len(n_groups) == 1:
            pass
        # chunk 1 exp (full width) + remaining chunks
        if PC > 1:
            exp_acc(1, 0, N1, NS)
        for c in range(2, PC):
            pm_tiles[c] = psum_mm.tile([128, 512 * ((N1 * 4 + 2047) // 2048)], f32, tag="pm", name=f"pm{c}")
            for si in range(NS):
                main_mm(c, si)
            exp_acc(c, 0, N1, NS - 1 + c)

        # ---- diag path (DVE, hidden under the exp phase)
        dgots = small.tile([128, PC], f32)
        for c in range(PC):
            dsq = scratch.tile([128, H], bf16, tag="dsq")
            nc.vector.scalar_tensor_tensor(
                out=dsq[:], in0=Gb[:, c, :], scalar=1.0, in1=Ef[:, c, :],
                op0=OP.mult, op1=OP.mult, accum_out=dgots[:, c : c + 1],
            )
        for c in range(PC):
            square_acc(nc.vector, Ef[:, c, :], sse[:, c : c + 1])
        rsqrt_of(sse[:], ln64_t)  # ACT slot; sse <- 64*rsqrt(sse)
        diag = small.tile([128, PC], f32)
        nc.vector.tensor_mul(diag[:], dgots[:], ssg[:])
        nc.vector.tensor_mul(diag[:], diag[:], sse[:])
        ed = small.tile([128, PC], f32)
        nc.scalar.activation(
            out=ed[:], in_=diag[:], func=AF.Exp, bias=cneg_t[:, 0:1], scale=1.0
        )

        # fold chunk-0 slab partials: S[NS-1] += S[NS-2] += ... (DVE, late)
        for i in range(1, NS):
            nc.vector.tensor_tensor(
                out=S[:, i : i + 1], in0=S[:, i - 1 : i], in1=S[:, i : i + 1], op=OP.add
            )

        # ---- per-row loss and masked partial sum
        t2 = small.tile([128, PC], f32)
        nc.vector.scalar_tensor_tensor(
            out=t2[:], in0=S[:, NS - 1 : NS - 1 + PC], scalar=corr_t[:, 0:1], in1=ed[:],
            op0=OP.add, op1=OP.add,
        )
        nc.scalar.activation(out=t2[:], in_=t2[:], func=AF.Ln, bias=eps_t[:, 0:1], scale=1.0)
        loss = small.tile([128, PC], f32)
        nc.vector.scalar_tensor_tensor(
            out=loss[:], in0=t2[:], scalar=CMAX, in1=diag[:],
            op0=OP.add, op1=OP.subtract,
        )
        lm = small.tile([128, PC], f32)
        part = small.tile([128, 1], f32)
        nc.vector.scalar_tensor_tensor(
            out=lm[:], in0=loss[:], scalar=1.0, in1=wt[:],
            op0=OP.mult, op1=OP.mult, accum_out=part[:],
        )
        nc.sync.dma_start(out=out[:], in_=part[:])
    if legalize:
        _legalize_waits(nc, max_waits=1)
    return nc


def _pad_rows(x: np.ndarray, n: int) -> np.ndarray:
    outp = np.zeros((n,) + x.shape[1:], dtype=x.dtype)
    outp[: x.shape[0]] = x
    return outp


def kernel(greek_embeds, english_embeds, labels):
    global LAST_RESULTS, LAST_SHAPES
    g = np.ascontiguousarray(np.asarray(greek_embeds, dtype=np.float32))
    e = np.ascontiguousarray(np.asarray(english_embeds, dtype=np.float32))
    lab = np.asarray(labels)
    B, P, Hh = g.shape
    assert Hh == H and B * 2 == N_CORES

    valid = lab != IGNORE_INDEX
    pos = valid & (lab == 1)
    neg = valid & (lab != 1)
    ok = (valid.sum(-1) >= 2) & pos.any(-1) & neg.any(-1)

    count = int(pos[ok].sum()) if ok.any() else 0
    if count == 0:
        return np.float32(0.0)

    pos_idx = [np.nonzero(pos[b])[0] if ok[b] else np.zeros(0, np.int64) for b in range(B)]
    neg_idx = [np.nonzero(neg[b])[0] if ok[b] else np.zeros(0, np.int64) for b in range(B)]
    halves = [np.array_split(pi, 2) for pi in pos_idx]

    np_max = max(len(halves[b][h]) for b in range(B) for h in range(2))
    nn_max = max(len(ni) for ni in neg_idx)
    P1 = max(128, ((np_max + 127) // 128) * 128)
    N1 = max(512, ((nn_max + 127) // 128) * 128)
    PC, NC = P1 // 128, N1 // 128

    E15 = np.float32(np.exp(np.float32(-CMAX)))
    bf16 = ml_dtypes.bfloat16
    fp8 = ml_dtypes.float8_e4m3
    in_maps = []
    for core in range(N_CORES):
        b, hf = core // 2, core % 2
        p_idx = halves[b][hf]
        n_idx = neg_idx[b]
        gr = _pad_rows(g[b][p_idx], P1)
        er = _pad_rows(e[b][n_idx], N1)
        epr = _pad_rows(e[b][p_idx], P1)
        w = np.zeros((128, PC), np.float32)
        npos = len(p_idx)
        for c in range(PC):
            w[: max(0, min(128, npos - c * 128)), c] = 1.0
        in_maps.append(
            {
                "g8t": np.ascontiguousarray(
                    (gr * FP8_SCALE).reshape(P1, 2, 128).transpose(2, 1, 0)
                ).astype(fp8),
                "gb": np.ascontiguousarray(
                    gr.astype(bf16).reshape(PC, 128, H).transpose(1, 0, 2)
                ),
                "ep": np.ascontiguousarray(
                    epr.astype(bf16).reshape(PC, 128, H).transpose(1, 0, 2)
                ),
                "en": np.ascontiguousarray(
                    er.astype(bf16).reshape(NC, 128, H).transpose(1, 0, 2)
                ),
                "wv": w,
                "corr": np.array([[-(N1 - len(n_idx)) * float(E15)]], np.float32),
            }
        )

    LAST_SHAPES = (P1, N1, dict(in_maps[0]))
    nc = _build_program(P1, N1)
    res = run_bass_kernel_spmd(nc, in_maps, list(range(N_CORES)), trace=TRACE)
    LAST_RESULTS = res
    total = sum(float(r["out"].sum()) for r in res.results)
    return np.float32(total / count)


# revision 7
# speedup vs baseline: 1.2052x; 1.0306x over previous
"""Contrastive loss kernel for Trainium2 (8 NeuronCores, Bass/Tile).

Strategy
--------
Only rows with label==1 (pos) contribute losses, and only columns with
label==0 (neg) plus the diagonal enter each row's logsumexp.  The host
computes the tiny index sets from `labels`, then each of the 8 cores
(2 per batch) receives its half of the batch's positive rows and all of
the batch's negative english rows, padded to uniform shapes (P1, N1).

Device pipeline (all reference FLOPs on device; host only gathers /
pads / transposes / casts):
  - g arrives twice: raw fp8e4 (x8) transposed [h%128, ktile, row] as
    the DoubleRow matmul stationary, and bf16 rows for norms + diag
    dots.  g's normalization folds into the exp's per-partition scale
    SG_p = 1/(64*T*|g_p|), so g is never scaled on device.
  - e-neg rows arrive bf16; per 128-row chunk: sum-of-squares (GPSIMD/
    DVE, accum), rsqrt via Ln/Exp (ACT), diag(s_q) built by an
    affine_select from a broadcast (GPSIMD), then one regular matmul
    per h-half computes transpose-and-scale fused:
    out[h, q] = e[q, h] * s_q.  PSUM -> SBUF fp8 copy on DVE/GPSIMD.
  - Main matmuls run in fp8e4 DoubleRow perf mode: one matmul per
    512-col slab contracts all H=256 (2 k-tiles) at 0.5 cycles/row.
  - exp(logit*SG_p - 15) runs in-place on the PSUM tile with accum_out
    giving per-row negative sums S.  Chunk 0 is exp'd per sub-slab so
    ACT starts as soon as the first e-chunk is through the pipe.
    Padded columns give exactly exp(-15), removed by a correction.
  - diag: bf16 row dots (DVE, accum) scaled by both rsqrt norms; the
    per-row loss is ln(exp(diag-15)+S+corr)+15-diag, masked and
    row-reduced; host sums the 8x128 partials and divides by count.
All ACT-table ops (Ln/Exp) interleave with the exp stream explicitly:
each engine queue executes in emission order, so every ACT op is
emitted at the point its inputs are expected to be ready.
"""

import sys

if "/opt/trn_rl_repo" not in sys.path:
    sys.path.insert(0, "/opt/trn_rl_repo")

from contextlib import ExitStack

import ml_dtypes
import numpy as np

import concourse.bass as bass
import concourse.tile as tile
from concourse import mybir
from concourse.bass_utils import run_bass_kernel_spmd
from concourse.masks import make_identity

TEMPERATURE = 0.07
IGNORE_INDEX = -100
CMAX = 15.0
H = 256
N_CORES = 8
FP8_SCALE = 8.0

LAST_RESULTS = None
LAST_SHAPES = None
TRACE = False


def _legalize_waits(nc: bass.Bass, max_waits: int = 1) -> None:
    """This container's walrus accepts at most one sync-wait per instruction
    (ACT structs especially); Tile can emit several.  Split the excess onto
    same-engine NoOps placed immediately before the instruction."""
    for bb in nc.main_func.blocks:
        new = []
        for ins in bb.instructions:
            si = ins.sync_info
            if si is not None and si.on_wait and len(si.on_wait) > max_waits:
                waits = list(si.on_wait)
                extra, keep = waits[:-max_waits], waits[-max_waits:]
                for i in range(0, len(extra), max_waits):
                    new.append(
                        mybir.InstNoOp(
                            name=nc.get_next_instruction_name(),
                            engine=ins.engine,
                            ins=[],
                            outs=[],
                            sync_info=mybir.SyncInfo(
                                on_wait=extra[i : i + max_waits], on_update=[]
                            ),
                            bass_nofuse=True,
                        )
                    )
                ins.sync_info = mybir.SyncInfo(
                    on_wait=keep, on_update=list(si.on_update or [])
                )
            new.append(ins)
        bb.instructions[:] = new


def _build_program(P1: int, N1: int, legalize: bool = True) -> bass.Bass:
    PC = P1 // 128
    NC = N1 // 128
    f32 = mybir.dt.float32
    bf16 = mybir.dt.bfloat16
    fp8 = mybir.dt.float8e4
    OP = mybir.AluOpType
    AF = mybir.ActivationFunctionType
    DR = mybir.MatmulPerfMode.DoubleRow

    nc = bass.Bass()
    g8t = nc.dram_tensor("g8t", [128, 2, P1], fp8, kind="ExternalInput")
    gb = nc.dram_tensor("gb", [128, PC, H], bf16, kind="ExternalInput")
    ep = nc.dram_tensor("ep", [128, PC, H], bf16, kind="ExternalInput")
    en = nc.dram_tensor("en", [128, NC, H], bf16, kind="ExternalInput")
    wv = nc.dram_tensor("wv", [128, PC], f32, kind="ExternalInput")
    corr = nc.dram_tensor("corr", [1, 1], f32, kind="ExternalInput")
    out = nc.dram_tensor("out", [128, 1], f32, kind="ExternalOutput")

    # exp blocks for chunk 0: sub-slab boundaries (bank-safe: within-bank or
    # bank-aligned).  [0:128] starts as early as possible.
    blk0 = [0, 128, 512, 1024, N1]
    blk0 = [b for b in blk0 if b < N1] + [N1]
    NB = len(blk0) - 1
    # e-chunk membership of each block
    blk_chunks = [list(range(blk0[i] // 128, (blk0[i + 1] + 127) // 128)) for i in range(NB)]
    # full-width matmul slabs for chunks >= 1
    slabs = [(s, min(s + 512, N1)) for s in range(0, N1, 512)]

    with tile.TileContext(nc) as tc, ExitStack() as ctx:
        persist = ctx.enter_context(tc.tile_pool(name="persist", bufs=1))
        small = ctx.enter_context(tc.tile_pool(name="small", bufs=1))
        scratch = ctx.enter_context(tc.tile_pool(name="scratch", bufs=4))
        dgpool = ctx.enter_context(tc.tile_pool(name="dgpool", bufs=3))
        psum_tp = ctx.enter_context(tc.tile_pool(name="psum_tp", bufs=2, space="PSUM"))
        psum_mm = ctx.enter_context(tc.tile_pool(name="psum_mm", bufs=2, space="PSUM"))

        # ---- constants
        eps_t = small.tile([128, 1], f32)
        nc.gpsimd.memset(eps_t[:], 1e-24)
        ln8_t = small.tile([128, 1], f32)
        nc.gpsimd.memset(ln8_t[:], float(np.log(FP8_SCALE)))
        mg_t = small.tile([128, 1], f32)
        nc.gpsimd.memset(mg_t[:], float(-np.log(FP8_SCALE * FP8_SCALE * TEMPERATURE)))
        ln64_t = small.tile([128, 1], f32)
        nc.gpsimd.memset(ln64_t[:], float(np.log(FP8_SCALE * FP8_SCALE)))
        cneg_t = small.tile([128, 1], f32)
        nc.gpsimd.memset(cneg_t[:], -CMAX)

        # ---- DMAs.  scalar queue: small first pieces then bulk; ACT's own
        # compute (dummy table load) is emitted after so it runs during the
        # DVE/GPSIMD square phase.  sync queue: e-chunk 0 first.
        EnA = persist.tile([128, 1, H], bf16)     # e chunk 0
        EnB = persist.tile([128, 3, H], bf16)     # e chunks 1-3
        EnC = persist.tile([128, 4, H], bf16, name="EnC") if NC > 4 else None
        EnD = persist.tile([128, NC - 8, H], bf16, name="EnD") if NC > 8 else None
        Gb0 = persist.tile([128, 1, H], bf16)     # g chunk 0
        GbR = persist.tile([128, PC - 1, H], bf16, name="GbR") if PC > 1 else None
        nc.sync.dma_start(out=EnA[:], in_=en[:, 0:1, :])
        nc.scalar.dma_start(out=Gb0[:], in_=gb[:, 0:1, :])
        nc.sync.dma_start(out=EnB[:], in_=en[:, 1:4, :])
        if EnC is not None:
            nc.scalar.dma_start(out=EnC[:], in_=en[:, 4:8, :])
        G8 = persist.tile([128, 2, P1], fp8)
        nc.sync.dma_start(out=G8[:], in_=g8t[:])
        if EnD is not None:
            nc.sync.dma_start(out=EnD[:], in_=en[:, 8:NC, :])
        if GbR is not None:
            nc.scalar.dma_start(out=GbR[:], in_=gb[:, 1:PC, :])
        Ef = persist.tile([128, PC, H], bf16)
        nc.scalar.dma_start(out=Ef[:], in_=ep[:])
        wt = small.tile([128, PC], f32)
        nc.sync.dma_start(out=wt[:], in_=wv[:])
        corr_t = small.tile([128, 1], f32)
        nc.sync.dma_start(out=corr_t[:], in_=corr[:].to_broadcast([128, 1]))

        # ACT table preload while DMAs/squares run
        dummy = small.tile([128, 1], f32)
        nc.scalar.activation(
            out=dummy[:], in_=eps_t[:], func=AF.Ln, bias=eps_t[:, 0:1], scale=1.0
        )

        ident = small.tile([128, 128], bf16)
        make_identity(nc, ident[:])

        ssn = small.tile([128, NC], f32)
        ssg = small.tile([128, PC], f32)
        sse = small.tile([128, PC], f32)
        eT8 = persist.tile([128, 2, N1], fp8)

        def en_src(c):
            if c == 0:
                return EnA[:, 0, :]
            if c < 4:
                return EnB[:, c - 1, :]
            if c < 8:
                return EnC[:, c - 4, :]
            return EnD[:, c - 8, :]

        def square_acc(eng, src, acc):
            sq = scratch.tile([128, H], bf16, tag="sq")
            eng.scalar_tensor_tensor(
                out=sq[:], in0=src, scalar=1.0, in1=src,
                op0=OP.mult, op1=OP.mult, accum_out=acc,
            )

        def rsqrt_of(ss, bias):
            nc.scalar.activation(out=ss, in_=ss, func=AF.Ln, bias=eps_t[:, 0:1], scale=1.0)
            nc.scalar.activation(out=ss, in_=ss, func=AF.Exp, bias=bias[:, 0:1], scale=-0.5)

        def diag_build(c):
            dg = dgpool.tile([128, 128], bf16, tag="dg")
            nc.gpsimd.affine_select(
                out=dg[:],
                in_=ssn[:, c : c + 1].to_broadcast([128, 128]),
                compare_op=OP.is_equal,
                fill=0.0, base=0, pattern=[[-1, 128]], channel_multiplier=1,
            )
            return dg

        def transp_copy(c, copy_eng):
            dg = diag_build(c)
            pt = psum_tp.tile([128, 2, 128], f32, tag="pt")
            for hk in range(2):
                nc.tensor.matmul(
                    pt[:, hk, :], en_src(c)[:, hk * 128 : (hk + 1) * 128], dg[:],
                    start=True, stop=True,
                )
            dst = eT8[:, :, c * 128 : (c + 1) * 128]
            if copy_eng is nc.scalar:
                nc.scalar.copy(out=dst, in_=pt[:])
            else:
                copy_eng.tensor_copy(out=dst, in_=pt[:])

        pm_cols = 512 * ((N1 * 4 + 2047) // 2048)
        pm_tiles = {}

        def new_pm(c):
            pm_tiles[c] = psum_mm.tile([128, pm_cols], f32, tag="pm", name=f"pm{c}")

        def main_mm(c, lo, hi):
            nc.tensor.matmul(
                pm_tiles[c][:, lo:hi],
                G8[:, :, c * 128 : (c + 1) * 128],
                eT8[:, :, lo:hi],
                start=True, stop=True, perf_mode=DR,
            )

        # S columns: chunk0 block partials [0..NB-1], then chunk c at NB-1+c
        S = small.tile([128, NB + PC - 1], f32)

        def exp_acc(c, lo, hi, s_col):
            nc.scalar.activation(
                out=pm_tiles[c][:, lo:hi], in_=pm_tiles[c][:, lo:hi], func=AF.Exp,
                bias=cneg_t[:, 0:1], scale=ssg[:, c : c + 1],
                accum_out=S[:, s_col : s_col + 1],
            )

        # ================= emission =================
        # DVE: e-sq chunk0 then 1-3; GPSIMD: g-sq 0 then e-sq 4+, g-sq rest
        square_acc(nc.vector, en_src(0), ssn[:, 0:1])
        square_acc(nc.gpsimd, Gb0[:, 0, :], ssg[:, 0:1])
        for c in range(1, 4):
            square_acc(nc.vector, en_src(c), ssn[:, c : c + 1])
        # ACT: rsqrt of e-chunk0 and g-chunk0 (first table ops after dummy)
        rsqrt_of(ssn[:, 0:1], ln8_t)
        rsqrt_of(ssg[:, 0:1], mg_t)
        # e chunk 0 through the pipe; first matmul + first exp block [0:128]
        transp_copy(0, nc.vector)
        new_pm(0)
        main_mm(0, 0, blk0[1])
        exp_acc(0, 0, blk0[1], 0)

        # squares for chunks 1-3 done on DVE; rsqrt + pipe + block [128:512]
        rsqrt_of(ssn[:, 1:4], ln8_t)
        for c in range(1, 4):
            transp_copy(c, nc.vector if c % 2 else nc.gpsimd)
        if NB > 1:
            main_mm(0, blk0[1], blk0[2])
            exp_acc(0, blk0[1], blk0[2], 1)

        # GPSIMD: e-sq 4-7, then 8+, then g-sq rest (emission order)
        for c in range(4, min(8, NC)):
            square_acc(nc.gpsimd, en_src(c), ssn[:, c : c + 1])
        for c in range(8, NC):
            square_acc(nc.gpsimd, en_src(c), ssn[:, c : c + 1])
        if PC > 1:
            for c in range(1, PC):
                square_acc(nc.gpsimd, GbR[:, c - 1, :], ssg[:, c : c + 1])

        if NC > 4:
            rsqrt_of(ssn[:, 4 : min(8, NC)], ln8_t)
            for c in range(4, min(8, NC)):
                transp_copy(c, nc.vector if c % 2 else nc.gpsimd)
            new_pm(1)
            main_mm(1, 0, 512)
            if NB > 2:
                main_mm(0, blk0[2], blk0[3])
                exp_acc(0, blk0[2], blk0[3], 2)
        if NC > 8:
            rsqrt_of(ssn[:, 8:NC], ln8_t)
            for c in range(8, NC):
                transp_copy(c, nc.vector if c % 2 else nc.gpsimd)
        if PC > 1:
            rsqrt_of(ssg[:, 1:PC], mg_t)
        if NB > 3:
            main_mm(0, blk0[3], blk0[4])
            exp_acc(0, blk0[3], blk0[4], 3)

        # diag-path squares/dots on DVE (run during the exp phase)
        dgots = small.tile([128, PC], f32)
        for c in range(PC):
            gsrc = Gb0[:, 0, :] if c == 0 else GbR[:, c - 1, :]
            dsq = scratch.tile([128, H], bf16, tag="dsq")
            nc.vector.scalar_tensor_tensor(
                out=dsq[:], in0=gsrc, scalar=1.0, in1=Ef[:, c, :],
                op0=OP.mult, op1=OP.mult, accum_out=dgots[:, c : c + 1],
            )
        for c in range(PC):
            square_acc(nc.vector, Ef[:, c, :], sse[:, c : c + 1])

        # chunk 1..: full-width matmuls + single exp each, with the remaining
        # ACT table ops slotted between exps at the point they become ready.
        for c in range(1, PC):
            if c not in pm_tiles:
                new_pm(c)
            for lo, hi in slabs:
                if c == 1 and lo == 0:
                    continue  # emitted early above
                main_mm(c, lo, hi)
            if c == 2:
                rsqrt_of(sse[:], ln64_t)  # sse <- 64*rsqrt(sse)
            if c == 3:
                # diag and its exp (dgots ready well before)
                diag = small.tile([128, PC], f32)
                nc.vector.tensor_mul(diag[:], dgots[:], ssg[:])
                nc.vector.tensor_mul(diag[:], diag[:], sse[:])
                ed = small.tile([128, PC], f32)
                nc.scalar.activation(
                    out=ed[:], in_=diag[:], func=AF.Exp, bias=cneg_t[:, 0:1], scale=1.0
                )
            exp_acc(c, 0, N1, NB - 1 + c)
        if PC <= 3:
            diag = small.tile([128, PC], f32)
            nc.vector.tensor_mul(diag[:], dgots[:], ssg[:])
            nc.vector.tensor_mul(diag[:], diag[:], sse[:])
            ed = small.tile([128, PC], f32)
            nc.scalar.activation(
                out=ed[:], in_=diag[:], func=AF.Exp, bias=cneg_t[:, 0:1], scale=1.0
            )

        # fold chunk-0 block partials (DVE; waits on exp0 accums)
        for i in range(1, NB):
            nc.vector.tensor_tensor(
                out=S[:, i : i + 1], in0=S[:, i - 1 : i], in1=S[:, i : i + 1], op=OP.add
            )

        # ---- per-row loss and masked partial sum
        t2 = small.tile([128, PC], f32)
        nc.vector.scalar_tensor_tensor(
            out=t2[:], in0=S[:, NB - 1 : NB - 1 + PC], scalar=corr_t[:, 0:1], in1=ed[:],
            op0=OP.add, op1=OP.add,
        )
        nc.scalar.activation(out=t2[:], in_=t2[:], func=AF.Ln, bias=eps_t[:, 0:1], scale=1.0)
        loss = small.tile([128, PC], f32)
        nc.vector.scalar_tensor_tensor(
            out=loss[:], in0=t2[:], scalar=CMAX, in1=diag[:],
            op0=OP.add, op1=OP.subtract,
        )
        lm = small.tile([128, PC], f32)
        part = small.tile([128, 1], f32)
        nc.vector.scalar_tensor_tensor(
            out=lm[:], in0=loss[:], scalar=1.0, in1=wt[:],
            op0=OP.mult, op1=OP.mult, accum_out=part[:],
        )
        nc.sync.dma_start(out=out[:], in_=part[:])
    if legalize:
        _legalize_waits(nc, max_waits=1)
    return nc


def _pad_rows(x: np.ndarray, n: int) -> np.ndarray:
    outp = np.zeros((n,) + x.shape[1:], dtype=x.dtype)
    outp[: x.shape[0]] = x
    return outp


def kernel(greek_embeds, english_embeds, labels):
    global LAST_RESULTS, LAST_SHAPES
    g = np.ascontiguousarray(np.asarray(greek_embeds, dtype=np.float32))
    e = np.ascontiguousarray(np.asarray(english_embeds, dtype=np.float32))
    lab = np.asarray(labels)
    B, P, Hh = g.shape
    assert Hh == H and B * 2 == N_CORES

    valid = lab != IGNORE_INDEX
    pos = valid & (lab == 1)
    neg = valid & (lab != 1)
    ok = (valid.sum(-1) >= 2) & pos.any(-1) & neg.any(-1)

    count = int(pos[ok].sum()) if ok.any() else 0
    if count == 0:
        return np.float32(0.0)

    pos_idx = [np.nonzero(pos[b])[0] if ok[b] else np.zeros(0, np.int64) for b in range(B)]
    neg_idx = [np.nonzero(neg[b])[0] if ok[b] else np.zeros(0, np.int64) for b in range(B)]
    halves = [np.array_split(pi, 2) for pi in pos_idx]

    np_max = max(len(halves[b][h]) for b in range(B) for h in range(2))
    nn_max = max(len(ni) for ni in neg_idx)
    P1 = max(128, ((np_max + 127) // 128) * 128)
    N1 = max(512, ((nn_max + 127) // 128) * 128)
    PC, NC = P1 // 128, N1 // 128

    E15 = np.float32(np.exp(np.float32(-CMAX)))
    bf16 = ml_dtypes.bfloat16
    fp8 = ml_dtypes.float8_e4m3
    in_maps = []
    for core in range(N_CORES):
        b, hf = core // 2, core % 2
        p_idx = halves[b][hf]
        n_idx = neg_idx[b]
        gr = _pad_rows(g[b][p_idx], P1)
        er = _pad_rows(e[b][n_idx], N1)
        epr = _pad_rows(e[b][p_idx], P1)
        w = np.zeros((128, PC), np.float32)
        npos = len(p_idx)
        for c in range(PC):
            w[: max(0, min(128, npos - c * 128)), c] = 1.0
        in_maps.append(
            {
                "g8t": np.ascontiguousarray(
                    (gr * FP8_SCALE).reshape(P1, 2, 128).transpose(2, 1, 0)
                ).astype(fp8),
                "gb": np.ascontiguousarray(
                    gr.astype(bf16).reshape(PC, 128, H).transpose(1, 0, 2)
                ),
                "ep": np.ascontiguousarray(
                    epr.astype(bf16).reshape(PC, 128, H).transpose(1, 0, 2)
                ),
                "en": np.ascontiguousarray(
                    er.astype(bf16).reshape(NC, 128, H).transpose(1, 0, 2)
                ),
                "wv": w,
                "corr": np.array([[-(N1 - len(n_idx)) * float(E15)]], np.float32),
            }
        )

    LAST_SHAPES = (P1, N1, dict(in_maps[0]))
    nc = _build_program(P1, N1)
    res = run_bass_kernel_spmd(nc, in_maps, list(range(N_CORES)), trace=TRACE)
    LAST_RESULTS = res
    total = sum(float(r["out"].sum()) for r in res.results)
    return np.float32(total / count)


# revision 9
# speedup vs baseline: 1.2412x; 1.0299x over previous
"""Contrastive loss kernel for Trainium2 (8 NeuronCores, Bass/Tile).

Strategy
--------
Only rows with label==1 (pos) contribute losses, and only columns with
label==0 (neg) plus the diagonal enter each row's logsumexp.  The host
computes the tiny index sets from `labels`, then each of the 8 cores
(2 per batch) receives its half of the batch's positive rows and all of
the batch's negative english rows, padded to uniform shapes (P1, N1).

Device pipeline (all reference FLOPs on device; host only gathers /
pads / transposes / casts):
  - g arrives twice: raw fp8e4 (x8) transposed [h%128, ktile, row] as
    the DoubleRow matmul stationary, and bf16 rows for norms + diag
    dots.  g's normalization folds into the exp's per-partition scale
    SG_p = 1/(64*T*|g_p|), so g is never scaled on device.
  - e-neg rows arrive bf16; per 128-row chunk: sum-of-squares (GPSIMD/
    DVE, accum), rsqrt via Ln/Exp (ACT), diag(s_q) built by an
    affine_select from a broadcast (GPSIMD), then one regular matmul
    per h-half computes transpose-and-scale fused:
    out[h, q] = e[q, h] * s_q.  PSUM -> SBUF fp8 copy on DVE/GPSIMD.
  - Main matmuls run in fp8e4 DoubleRow perf mode: one matmul per
    512-col slab contracts all H=256 (2 k-tiles) at 0.5 cycles/row.
  - exp(logit*SG_p - 15) runs in-place on the PSUM tile with accum_out
    giving per-row negative sums S.  Chunk 0 is exp'd per sub-slab so
    ACT starts as soon as the first e-chunk is through the pipe.
    Padded columns give exactly exp(-15), removed by a correction.
  - diag: bf16 row dots (DVE, accum) scaled by both rsqrt norms; the
    per-row loss is ln(exp(diag-15)+S+corr)+15-diag, masked and
    row-reduced; host sums the 8x128 partials and divides by count.
All ACT-table ops (Ln/Exp) interleave with the exp stream explicitly:
each engine queue executes in emission order, so every ACT op is
emitted at the point its inputs are expected to be ready.
"""

import sys

if "/opt/trn_rl_repo" not in sys.path:
    sys.path.insert(0, "/opt/trn_rl_repo")

from contextlib import ExitStack

import ml_dtypes
import numpy as np

import concourse.bass as bass
import concourse.tile as tile
from concourse import mybir
from concourse.bass_utils import run_bass_kernel_spmd
from concourse.masks import make_identity

TEMPERATURE = 0.07
IGNORE_INDEX = -100
CMAX = 15.0
H = 256
N_CORES = 8
FP8_SCALE = 8.0

LAST_RESULTS = None
LAST_SHAPES = None
TRACE = False


def _legalize_waits(nc: bass.Bass, max_waits: int = 1) -> None:
    """This container's walrus accepts at most one sync-wait per instruction
    (ACT structs especially); Tile can emit several.  Split the excess onto
    same-engine NoOps placed immediately before the instruction."""
    for bb in nc.main_func.blocks:
        new = []
        for ins in bb.instructions:
            si = ins.sync_info
            if si is not None and si.on_wait and len(si.on_wait) > max_waits:
                waits = list(si.on_wait)
                extra, keep = waits[:-max_waits], waits[-max_waits:]
                for i in range(0, len(extra), max_waits):
                    new.append(
                        mybir.InstNoOp(
                            name=nc.get_next_instruction_name(),
                            engine=ins.engine,
                            ins=[],
                            outs=[],
                            sync_info=mybir.SyncInfo(
                                on_wait=extra[i : i + max_waits], on_update=[]
                            ),
                            bass_nofuse=True,
                        )
                    )
                ins.sync_info = mybir.SyncInfo(
                    on_wait=keep, on_update=list(si.on_update or [])
                )
            new.append(ins)
        bb.instructions[:] = new


def _build_program(P1: int, N1: int, legalize: bool = True) -> bass.Bass:
    PC = P1 // 128
    NC = N1 // 128
    f32 = mybir.dt.float32
    bf16 = mybir.dt.bfloat16
    fp8 = mybir.dt.float8e4
    OP = mybir.AluOpType
    AF = mybir.ActivationFunctionType
    DR = mybir.MatmulPerfMode.DoubleRow

    nc = bass.Bass()
    g8t = nc.dram_tensor("g8t", [128, 2, P1], fp8, kind="ExternalInput")
    gb = nc.dram_tensor("gb", [128, PC, H], bf16, kind="ExternalInput")
    ep = nc.dram_tensor("ep", [128, PC, H], bf16, kind="ExternalInput")
    en = nc.dram_tensor("en", [128, NC, H], bf16, kind="ExternalInput")
    wv = nc.dram_tensor("wv", [128, PC], f32, kind="ExternalInput")
    corr = nc.dram_tensor("corr", [1, 1], f32, kind="ExternalInput")
    out = nc.dram_tensor("out", [128, 1], f32, kind="ExternalOutput")

    # exp blocks for chunk 0: sub-slab boundaries (bank-safe: within-bank or
    # bank-aligned).  [0:128] starts as early as possible.
    blk0 = [0, 512, 1024, N1]
    blk0 = [b for b in blk0 if b < N1] + [N1]
    NB = len(blk0) - 1
    # e-chunk membership of each block
    blk_chunks = [list(range(blk0[i] // 128, (blk0[i + 1] + 127) // 128)) for i in range(NB)]
    # full-width matmul slabs for chunks >= 1
    slabs = [(s, min(s + 512, N1)) for s in range(0, N1, 512)]

    with tile.TileContext(nc) as tc, ExitStack() as ctx:
        persist = ctx.enter_context(tc.tile_pool(name="persist", bufs=1))
        small = ctx.enter_context(tc.tile_pool(name="small", bufs=1))
        scratch = ctx.enter_context(tc.tile_pool(name="scratch", bufs=4))
        dgpool = ctx.enter_context(tc.tile_pool(name="dgpool", bufs=3))
        psum_tp = ctx.enter_context(tc.tile_pool(name="psum_tp", bufs=2, space="PSUM"))
        psum_mm = ctx.enter_context(tc.tile_pool(name="psum_mm", bufs=2, space="PSUM"))

        # ---- constants
        eps_t = small.tile([128, 1], f32)
        nc.gpsimd.memset(eps_t[:], 1e-24)
        ln8_t = small.tile([128, 1], f32)
        nc.gpsimd.memset(ln8_t[:], float(np.log(FP8_SCALE)))
        mg_t = small.tile([128, 1], f32)
        nc.gpsimd.memset(mg_t[:], float(-np.log(FP8_SCALE * FP8_SCALE * TEMPERATURE)))
        ln64_t = small.tile([128, 1], f32)
        nc.gpsimd.memset(ln64_t[:], float(np.log(FP8_SCALE * FP8_SCALE)))
        cneg_t = small.tile([128, 1], f32)
        nc.gpsimd.memset(cneg_t[:], -CMAX)

        # ---- DMAs.  scalar queue: small first pieces then bulk; ACT's own
        # compute (dummy table load) is emitted after so it runs during the
        # DVE/GPSIMD square phase.  sync queue: e-chunk 0 first.
        EnA = persist.tile([128, 1, H], bf16)     # e chunk 0
        EnB = persist.tile([128, 3, H], bf16)     # e chunks 1-3
        EnC = persist.tile([128, 4, H], bf16, name="EnC") if NC > 4 else None
        EnD = persist.tile([128, NC - 8, H], bf16, name="EnD") if NC > 8 else None
        Gb0 = persist.tile([128, 1, H], bf16)     # g chunk 0
        GbR = persist.tile([128, PC - 1, H], bf16, name="GbR") if PC > 1 else None
        nc.scalar.dma_start(out=Gb0[:], in_=gb[:, 0:1, :])
        # ACT table preload immediately after the one small scalar-queue DMA;
        # every other DMA goes on sync so ACT's SEQ is free for table ops.
        dummy = small.tile([128, 1], f32)
        nc.scalar.activation(
            out=dummy[:], in_=eps_t[:], func=AF.Ln, bias=eps_t[:, 0:1], scale=1.0
        )
        nc.sync.dma_start(out=EnA[:], in_=en[:, 0:1, :])
        nc.sync.dma_start(out=EnB[:], in_=en[:, 1:4, :])
        if EnC is not None:
            nc.sync.dma_start(out=EnC[:], in_=en[:, 4:8, :])
        G8 = persist.tile([128, 2, P1], fp8)
        nc.sync.dma_start(out=G8[:], in_=g8t[:])
        if EnD is not None:
            nc.sync.dma_start(out=EnD[:], in_=en[:, 8:NC, :])
        if GbR is not None:
            nc.sync.dma_start(out=GbR[:], in_=gb[:, 1:PC, :])
        Ef = persist.tile([128, PC, H], bf16)
        nc.sync.dma_start(out=Ef[:], in_=ep[:])
        wt = small.tile([128, PC], f32)
        nc.sync.dma_start(out=wt[:], in_=wv[:])
        corr_t = small.tile([128, 1], f32)
        nc.sync.dma_start(out=corr_t[:], in_=corr[:].to_broadcast([128, 1]))

        ident = small.tile([128, 128], bf16)
        make_identity(nc, ident[:])

        ssn = small.tile([128, NC], f32)
        ssg = small.tile([128, PC], f32)
        sse = small.tile([128, PC], f32)
        eT8 = persist.tile([128, 2, N1], fp8)

        def en_src(c):
            if c == 0:
                return EnA[:, 0, :]
            if c < 4:
                return EnB[:, c - 1, :]
            if c < 8:
                return EnC[:, c - 4, :]
            return EnD[:, c - 8, :]

        def square_acc(eng, src, acc):
            sq = scratch.tile([128, H], bf16, tag="sq")
            eng.scalar_tensor_tensor(
                out=sq[:], in0=src, scalar=1.0, in1=src,
                op0=OP.mult, op1=OP.mult, accum_out=acc,
            )

        def rsqrt_of(ss, bias):
            nc.scalar.activation(out=ss, in_=ss, func=AF.Ln, bias=eps_t[:, 0:1], scale=1.0)
            nc.scalar.activation(out=ss, in_=ss, func=AF.Exp, bias=bias[:, 0:1], scale=-0.5)

        def diag_build(c):
            dg = dgpool.tile([128, 128], bf16, tag="dg")
            nc.gpsimd.affine_select(
                out=dg[:],
                in_=ssn[:, c : c + 1].to_broadcast([128, 128]),
                compare_op=OP.is_equal,
                fill=0.0, base=0, pattern=[[-1, 128]], channel_multiplier=1,
            )
            return dg

        def transp_copy(c, copy_eng):
            dg = diag_build(c)
            pt = psum_tp.tile([128, 2, 128], f32, tag="pt")
            for hk in range(2):
                nc.tensor.matmul(
                    pt[:, hk, :], en_src(c)[:, hk * 128 : (hk + 1) * 128], dg[:],
                    start=True, stop=True,
                )
            dst = eT8[:, :, c * 128 : (c + 1) * 128]
            if copy_eng is nc.scalar:
                nc.scalar.copy(out=dst, in_=pt[:])
            else:
                copy_eng.tensor_copy(out=dst, in_=pt[:])

        pm_cols = 512 * ((N1 * 4 + 2047) // 2048)
        pm_tiles = {}

        def new_pm(c):
            pm_tiles[c] = psum_mm.tile([128, pm_cols], f32, tag="pm", name=f"pm{c}")

        def main_mm(c, lo, hi):
            nc.tensor.matmul(
                pm_tiles[c][:, lo:hi],
                G8[:, :, c * 128 : (c + 1) * 128],
                eT8[:, :, lo:hi],
                start=True, stop=True, perf_mode=DR,
            )

        # S columns: chunk0 block partials [0..NB-1], then chunk c at NB-1+c
        S = small.tile([128, NB + PC - 1], f32)

        def exp_acc(c, lo, hi, s_col):
            nc.scalar.activation(
                out=pm_tiles[c][:, lo:hi], in_=pm_tiles[c][:, lo:hi], func=AF.Exp,
                bias=cneg_t[:, 0:1], scale=ssg[:, c : c + 1],
                accum_out=S[:, s_col : s_col + 1],
            )

        # ================= emission =================
        # DVE: e-sq chunk0 then 1-3; GPSIMD: g-sq 0 then e-sq 4+, g-sq rest
        square_acc(nc.gpsimd, Gb0[:, 0, :], ssg[:, 0:1])
        for c in range(0, 4):
            square_acc(nc.vector, en_src(c), ssn[:, c : c + 1])
        # ACT: rsqrt of e-chunks 0-3 and g-chunk0 (first table ops)
        rsqrt_of(ssn[:, 0:4], ln8_t)
        rsqrt_of(ssg[:, 0:1], mg_t)
        for c in range(0, 4):
            transp_copy(c, nc.vector if c % 2 else nc.gpsimd)
        new_pm(0)
        main_mm(0, 0, blk0[1])
        exp_acc(0, 0, blk0[1], 0)

        # GPSIMD: e-sq 4-7, then 8+, then g-sq rest (emission order)
        for c in range(4, min(8, NC)):
            square_acc(nc.gpsimd, en_src(c), ssn[:, c : c + 1])
        for c in range(8, NC):
            square_acc(nc.gpsimd, en_src(c), ssn[:, c : c + 1])
        if PC > 1:
            for c in range(1, PC):
                square_acc(nc.gpsimd, GbR[:, c - 1, :], ssg[:, c : c + 1])

        if NC > 4:
            rsqrt_of(ssn[:, 4 : min(8, NC)], ln8_t)
            for c in range(4, min(8, NC)):
                transp_copy(c, nc.vector if c % 2 else nc.gpsimd)
            new_pm(1)
            main_mm(1, 0, 512)
            if NB > 1:
                main_mm(0, blk0[1], blk0[2])
                exp_acc(0, blk0[1], blk0[2], 1)
        if NC > 8:
            rsqrt_of(ssn[:, 8:NC], ln8_t)
            for c in range(8, NC):
                transp_copy(c, nc.vector if c % 2 else nc.gpsimd)
        if PC > 1:
            rsqrt_of(ssg[:, 1:PC], mg_t)
        if NB > 2:
            main_mm(0, blk0[2], blk0[3])
            exp_acc(0, blk0[2], blk0[3], 2)

        # diag-path squares/dots on DVE (run during the exp phase)
        dgots = small.tile([128, PC], f32)
        for c in range(PC):
            gsrc = Gb0[:, 0, :] if c == 0 else GbR[:, c - 1, :]
            dsq = scratch.tile([128, H], bf16, tag="dsq")
            nc.vector.scalar_tensor_tensor(
                out=dsq[:], in0=gsrc, scalar=1.0, in1=Ef[:, c, :],
                op0=OP.mult, op1=OP.mult, accum_out=dgots[:, c : c + 1],
            )
        for c in range(PC):
            square_acc(nc.vector, Ef[:, c, :], sse[:, c : c + 1])

        # chunk 1..: full-width matmuls + single exp each, with the remaining
        # ACT table ops slotted between exps at the point they become ready.
        for c in range(1, PC):
            if c not in pm_tiles:
                new_pm(c)
            for lo, hi in slabs:
                if c == 1 and lo == 0:
                    continue  # emitted early above
                main_mm(c, lo, hi)
            if c == 2:
                rsqrt_of(sse[:], ln64_t)  # sse <- 64*rsqrt(sse)
            if c == 3:
                # diag and its exp (dgots ready well before)
                diag = small.tile([128, PC], f32)
                nc.vector.tensor_mul(diag[:], dgots[:], ssg[:])
                nc.vector.tensor_mul(diag[:], diag[:], sse[:])
                ed = small.tile([128, PC], f32)
                nc.scalar.activation(
                    out=ed[:], in_=diag[:], func=AF.Exp, bias=cneg_t[:, 0:1], scale=1.0
                )
            exp_acc(c, 0, N1, NB - 1 + c)
        if PC <= 3:
            diag = small.tile([128, PC], f32)
            nc.vector.tensor_mul(diag[:], dgots[:], ssg[:])
            nc.vector.tensor_mul(diag[:], diag[:], sse[:])
            ed = small.tile([128, PC], f32)
            nc.scalar.activation(
                out=ed[:], in_=diag[:], func=AF.Exp, bias=cneg_t[:, 0:1], scale=1.0
            )

        # fold chunk-0 block partials (DVE; waits on exp0 accums)
        for i in range(1, NB):
            nc.vector.tensor_tensor(
                out=S[:, i : i + 1], in0=S[:, i - 1 : i], in1=S[:, i : i + 1], op=OP.add
            )

        # ---- per-row loss and masked partial sum
        t2 = small.tile([128, PC], f32)
        nc.vector.scalar_tensor_tensor(
            out=t2[:], in0=S[:, NB - 1 : NB - 1 + PC], scalar=corr_t[:, 0:1], in1=ed[:],
            op0=OP.add, op1=OP.add,
        )
        nc.scalar.activation(out=t2[:], in_=t2[:], func=AF.Ln, bias=eps_t[:, 0:1], scale=1.0)
        loss = small.tile([128, PC], f32)
        nc.vector.scalar_tensor_tensor(
            out=loss[:], in0=t2[:], scalar=CMAX, in1=diag[:],
            op0=OP.add, op1=OP.subtract,
        )
        lm = small.tile([128, PC], f32)
        part = small.tile([128, 1], f32)
        nc.vector.scalar_tensor_tensor(
            out=lm[:], in0=loss[:], scalar=1.0, in1=wt[:],
            op0=OP.mult, op1=OP.mult, accum_out=part[:],
        )
        nc.sync.dma_start(out=out[:], in_=part[:])
    if legalize:
        _legalize_waits(nc, max_waits=1)
    return nc


def _pad_rows(x: np.ndarray, n: int) -> np.ndarray:
    outp = np.zeros((n,) + x.shape[1:], dtype=x.dtype)
    outp[: x.shape[0]] = x
    return outp


def kernel(greek_embeds, english_embeds, labels):
    global LAST_RESULTS, LAST_SHAPES
    g = np.ascontiguousarray(np.asarray(greek_embeds, dtype=np.float32))
    e = np.ascontiguousarray(np.asarray(english_embeds, dtype=np.float32))
    lab = np.asarray(labels)
    B, P, Hh = g.shape
    assert Hh == H and B * 2 == N_CORES

    valid = lab != IGNORE_INDEX
    pos = valid & (lab == 1)
    neg = valid & (lab != 1)
    ok = (valid.sum(-1) >= 2) & pos.any(-1) & neg.any(-1)

    count = int(pos[ok].sum()) if ok.any() else 0
    if count == 0:
        return np.float32(0.0)

    pos_idx = [np.nonzero(pos[b])[0] if ok[b] else np.zeros(0, np.int64) for b in range(B)]
    neg_idx = [np.nonzero(neg[b])[0] if ok[b] else np.zeros(0, np.int64) for b in range(B)]
    halves = [np.array_split(pi, 2) for pi in pos_idx]

    np_max = max(len(halves[b][h]) for b in range(B) for h in range(2))
    nn_max = max(len(ni) for ni in neg_idx)
    P1 = max(128, ((np_max + 127) // 128) * 128)
    N1 = max(512, ((nn_max + 127) // 128) * 128)
    PC, NC = P1 // 128, N1 // 128

    E15 = np.float32(np.exp(np.float32(-CMAX)))
    bf16 = ml_dtypes.bfloat16
    fp8 = ml_dtypes.float8_e4m3
    in_maps = []
    for core in range(N_CORES):
        b, hf = core // 2, core % 2
        p_idx = halves[b][hf]
        n_idx = neg_idx[b]
        gr = _pad_rows(g[b][p_idx], P1)
        er = _pad_rows(e[b][n_idx], N1)
        epr = _pad_rows(e[b][p_idx], P1)
        w = np.zeros((128, PC), np.float32)
        npos = len(p_idx)
        for c in range(PC):
            w[: max(0, min(128, npos - c * 128)), c] = 1.0
        in_maps.append(
            {
                "g8t": np.ascontiguousarray(
                    (gr * FP8_SCALE).reshape(P1, 2, 128).transpose(2, 1, 0)
                ).astype(fp8),
                "gb": np.ascontiguousarray(
                    gr.astype(bf16).reshape(PC, 128, H).transpose(1, 0, 2)
                ),
                "ep": np.ascontiguousarray(
                    epr.astype(bf16).reshape(PC, 128, H).transpose(1, 0, 2)
                ),
                "en": np.ascontiguousarray(
                    er.astype(bf16).reshape(NC, 128, H).transpose(1, 0, 2)
                ),
                "wv": w,
                "corr": np.array([[-(N1 - len(n_idx)) * float(E15)]], np.float32),
            }
        )

    LAST_SHAPES = (P1, N1, dict(in_maps[0]))
    nc = _build_program(P1, N1)
    res = run_bass_kernel_spmd(nc, in_maps, list(range(N_CORES)), trace=TRACE)
    LAST_RESULTS = res
    total = sum(float(r["out"].sum()) for r in res.results)
    return np.float32(total / count)


# revision 10
# speedup vs baseline: 1.2516x; 1.0083x over previous
"""Contrastive loss kernel for Trainium2 (8 NeuronCores, Bass/Tile).

Strategy
--------
Only rows with label==1 (pos) contribute losses, and only columns with
label==0 (neg) plus the diagonal enter each row's logsumexp.  The host
computes the tiny index sets from `labels`, then each of the 8 cores
(2 per batch) receives its half of the batch's positive rows and all of
the batch's negative english rows, padded to uniform shapes (P1, N1).

Device pipeline (all reference FLOPs on device; host only gathers /
pads / transposes / casts):
  - g arrives twice: raw fp8e4 (x8) transposed [h%128, ktile, row] as
    the DoubleRow matmul stationary, and bf16 rows for norms + diag
    dots.  g's normalization folds into the exp's per-partition scale
    SG_p = 1/(64*T*|g_p|), so g is never scaled on device.
  - e-neg rows arrive bf16; per 128-row chunk: sum-of-squares (GPSIMD/
    DVE, accum), rsqrt via Ln/Exp (ACT), diag(s_q) built by an
    affine_select from a broadcast (GPSIMD), then one regular matmul
    per h-half computes transpose-and-scale fused:
    out[h, q] = e[q, h] * s_q.  PSUM -> SBUF fp8 copy on DVE/GPSIMD.
  - Main matmuls run in fp8e4 DoubleRow perf mode: one matmul per
    512-col slab contracts all H=256 (2 k-tiles) at 0.5 cycles/row.
  - exp(logit*SG_p - 15) runs in-place on the PSUM tile with accum_out
    giving per-row negative sums S.  Chunk 0 is exp'd per sub-slab so
    ACT starts as soon as the first e-chunk is through the pipe.
    Padded columns give exactly exp(-15), removed by a correction.
  - diag: bf16 row dots (DVE, accum) scaled by both rsqrt norms; the
    per-row loss is ln(exp(diag-15)+S+corr)+15-diag, masked and
    row-reduced; host sums the 8x128 partials and divides by count.
All ACT-table ops (Ln/Exp) interleave with the exp stream explicitly:
each engine queue executes in emission order, so every ACT op is
emitted at the point its inputs are expected to be ready.
"""

import sys

if "/opt/trn_rl_repo" not in sys.path:
    sys.path.insert(0, "/opt/trn_rl_repo")

from contextlib import ExitStack

import ml_dtypes
import numpy as np

import concourse.bass as bass
import concourse.tile as tile
from concourse import mybir
from concourse.bass_utils import run_bass_kernel_spmd
from concourse.masks import make_identity

TEMPERATURE = 0.07
IGNORE_INDEX = -100
CMAX = 15.0
H = 256
N_CORES = 8
FP8_SCALE = 8.0

LAST_RESULTS = None
LAST_SHAPES = None
TRACE = False


def _legalize_waits(nc: bass.Bass, max_waits: int = 1) -> None:
    """This container's walrus accepts at most one sync-wait per instruction
    (ACT structs especially); Tile can emit several.  Split the excess onto
    same-engine NoOps placed immediately before the instruction."""
    for bb in nc.main_func.blocks:
        new = []
        for ins in bb.instructions:
            si = ins.sync_info
            if si is not None and si.on_wait and len(si.on_wait) > max_waits:
                waits = list(si.on_wait)
                extra, keep = waits[:-max_waits], waits[-max_waits:]
                for i in range(0, len(extra), max_waits):
                    new.append(
                        mybir.InstNoOp(
                            name=nc.get_next_instruction_name(),
                            engine=ins.engine,
                            ins=[],
                            outs=[],
                            sync_info=mybir.SyncInfo(
                                on_wait=extra[i : i + max_waits], on_update=[]
                            ),
                            bass_nofuse=True,
                        )
                    )
                ins.sync_info = mybir.SyncInfo(
                    on_wait=keep, on_update=list(si.on_update or [])
                )
            new.append(ins)
        bb.instructions[:] = new


def _build_program(P1: int, N1: int, legalize: bool = True) -> bass.Bass:
    PC = P1 // 128
    NC = N1 // 128
    f32 = mybir.dt.float32
    bf16 = mybir.dt.bfloat16
    fp8 = mybir.dt.float8e4
    OP = mybir.AluOpType
    AF = mybir.ActivationFunctionType
    DR = mybir.MatmulPerfMode.DoubleRow

    nc = bass.Bass()
    g8t = nc.dram_tensor("g8t", [128, 2, P1], fp8, kind="ExternalInput")
    gb = nc.dram_tensor("gb", [128, PC, H], bf16, kind="ExternalInput")
    ep = nc.dram_tensor("ep", [128, PC, H], bf16, kind="ExternalInput")
    en = nc.dram_tensor("en", [128, NC, H], bf16, kind="ExternalInput")
    wv = nc.dram_tensor("wv", [128, PC], f32, kind="ExternalInput")
    corr = nc.dram_tensor("corr", [1, 1], f32, kind="ExternalInput")
    out = nc.dram_tensor("out", [128, 1], f32, kind="ExternalOutput")

    # exp blocks for chunk 0: sub-slab boundaries (bank-safe: within-bank or
    # bank-aligned).  [0:128] starts as early as possible.
    blk0 = [0, 512, 1024, N1]
    blk0 = [b for b in blk0 if b < N1] + [N1]
    NB = len(blk0) - 1
    # e-chunk membership of each block
    blk_chunks = [list(range(blk0[i] // 128, (blk0[i + 1] + 127) // 128)) for i in range(NB)]
    # full-width matmul slabs for chunks >= 1
    slabs = [(s, min(s + 512, N1)) for s in range(0, N1, 512)]

    with tile.TileContext(nc) as tc, ExitStack() as ctx:
        persist = ctx.enter_context(tc.tile_pool(name="persist", bufs=1))
        small = ctx.enter_context(tc.tile_pool(name="small", bufs=1))
        scratch = ctx.enter_context(tc.tile_pool(name="scratch", bufs=4))
        dgpool = ctx.enter_context(tc.tile_pool(name="dgpool", bufs=3))
        psum_tp = ctx.enter_context(tc.tile_pool(name="psum_tp", bufs=2, space="PSUM"))
        psum_mm = ctx.enter_context(tc.tile_pool(name="psum_mm", bufs=2, space="PSUM"))

        # ---- constants
        eps_t = small.tile([128, 1], f32)
        nc.gpsimd.memset(eps_t[:], 1e-24)
        ln8_t = small.tile([128, 1], f32)
        nc.gpsimd.memset(ln8_t[:], float(np.log(FP8_SCALE)))
        mg_t = small.tile([128, 1], f32)
        nc.gpsimd.memset(mg_t[:], float(-np.log(FP8_SCALE * FP8_SCALE * TEMPERATURE)))
        ln64_t = small.tile([128, 1], f32)
        nc.gpsimd.memset(ln64_t[:], float(np.log(FP8_SCALE * FP8_SCALE)))
        cneg_t = small.tile([128, 1], f32)
        nc.gpsimd.memset(cneg_t[:], -CMAX)

        # ---- DMAs.  scalar queue: small first pieces then bulk; ACT's own
        # compute (dummy table load) is emitted after so it runs during the
        # DVE/GPSIMD square phase.  sync queue: e-chunk 0 first.
        EnA = persist.tile([128, 1, H], bf16)     # e chunk 0
        EnB = persist.tile([128, 3, H], bf16)     # e chunks 1-3
        EnC = persist.tile([128, 4, H], bf16, name="EnC") if NC > 4 else None
        EnD = persist.tile([128, NC - 8, H], bf16, name="EnD") if NC > 8 else None
        Gb0 = persist.tile([128, 1, H], bf16)     # g chunk 0
        GbR = persist.tile([128, PC - 1, H], bf16, name="GbR") if PC > 1 else None
        nc.scalar.dma_start(out=Gb0[:], in_=gb[:, 0:1, :])
        # ACT table preload immediately after the one small scalar-queue DMA;
        # every other DMA goes on sync so ACT's SEQ is free for table ops.
        dummy = small.tile([128, 1], f32)
        nc.scalar.activation(
            out=dummy[:], in_=eps_t[:], func=AF.Ln, bias=eps_t[:, 0:1], scale=1.0
        )
        nc.sync.dma_start(out=EnA[:], in_=en[:, 0:1, :])
        nc.sync.dma_start(out=EnB[:], in_=en[:, 1:4, :])
        if GbR is not None:
            nc.sync.dma_start(out=GbR[:], in_=gb[:, 1:PC, :])
        if EnC is not None:
            nc.sync.dma_start(out=EnC[:], in_=en[:, 4:8, :])
        G8 = persist.tile([128, 2, P1], fp8)
        nc.sync.dma_start(out=G8[:], in_=g8t[:])
        if EnD is not None:
            nc.sync.dma_start(out=EnD[:], in_=en[:, 8:NC, :])
        Ef = persist.tile([128, PC, H], bf16)
        nc.sync.dma_start(out=Ef[:], in_=ep[:])
        wt = small.tile([128, PC], f32)
        nc.sync.dma_start(out=wt[:], in_=wv[:])
        corr_t = small.tile([128, 1], f32)
        nc.sync.dma_start(out=corr_t[:], in_=corr[:].to_broadcast([128, 1]))

        ident = small.tile([128, 128], bf16)
        make_identity(nc, ident[:])

        ssn = small.tile([128, NC], f32)
        ssg = small.tile([128, PC], f32)
        sse = small.tile([128, PC], f32)
        eT8 = persist.tile([128, 2, N1], fp8)

        def en_src(c):
            if c == 0:
                return EnA[:, 0, :]
            if c < 4:
                return EnB[:, c - 1, :]
            if c < 8:
                return EnC[:, c - 4, :]
            return EnD[:, c - 8, :]

        def square_acc(eng, src, acc):
            sq = scratch.tile([128, H], bf16, tag="sq")
            eng.scalar_tensor_tensor(
                out=sq[:], in0=src, scalar=1.0, in1=src,
                op0=OP.mult, op1=OP.mult, accum_out=acc,
            )

        def rsqrt_of(ss, bias):
            nc.scalar.activation(out=ss, in_=ss, func=AF.Ln, bias=eps_t[:, 0:1], scale=1.0)
            nc.scalar.activation(out=ss, in_=ss, func=AF.Exp, bias=bias[:, 0:1], scale=-0.5)

        def diag_build(c):
            dg = dgpool.tile([128, 128], bf16, tag="dg")
            nc.gpsimd.affine_select(
                out=dg[:],
                in_=ssn[:, c : c + 1].to_broadcast([128, 128]),
                compare_op=OP.is_equal,
                fill=0.0, base=0, pattern=[[-1, 128]], channel_multiplier=1,
            )
            return dg

        def transp_copy(c, copy_eng):
            dg = diag_build(c)
            pt = psum_tp.tile([128, 2, 128], f32, tag="pt")
            for hk in range(2):
                nc.tensor.matmul(
                    pt[:, hk, :], en_src(c)[:, hk * 128 : (hk + 1) * 128], dg[:],
                    start=True, stop=True,
                )
            dst = eT8[:, :, c * 128 : (c + 1) * 128]
            if copy_eng is nc.scalar:
                nc.scalar.copy(out=dst, in_=pt[:])
            else:
                copy_eng.tensor_copy(out=dst, in_=pt[:])

        pm_cols = 512 * ((N1 * 4 + 2047) // 2048)
        pm_tiles = {}

        def new_pm(c):
            pm_tiles[c] = psum_mm.tile([128, pm_cols], f32, tag="pm", name=f"pm{c}")

        def main_mm(c, lo, hi):
            nc.tensor.matmul(
                pm_tiles[c][:, lo:hi],
                G8[:, :, c * 128 : (c + 1) * 128],
                eT8[:, :, lo:hi],
                start=True, stop=True, perf_mode=DR,
            )

        # S columns: chunk0 block partials [0..NB-1], then chunk c at NB-1+c
        S = small.tile([128, NB + PC - 1], f32)

        def exp_acc(c, lo, hi, s_col):
            nc.scalar.activation(
                out=pm_tiles[c][:, lo:hi], in_=pm_tiles[c][:, lo:hi], func=AF.Exp,
                bias=cneg_t[:, 0:1], scale=ssg[:, c : c + 1],
                accum_out=S[:, s_col : s_col + 1],
            )

        # ================= emission =================
        # DVE: e-sq chunk0 then 1-3; GPSIMD: g-sq 0 then e-sq 4+, g-sq rest
        square_acc(nc.gpsimd, Gb0[:, 0, :], ssg[:, 0:1])
        for c in range(0, 2):
            square_acc(nc.vector, en_src(c), ssn[:, c : c + 1])
        for c in range(2, 4):
            square_acc(nc.gpsimd, en_src(c), ssn[:, c : c + 1])
        # ACT: rsqrt of e-chunks 0-3 and g-chunk0 (first table ops)
        rsqrt_of(ssn[:, 0:4], ln8_t)
        rsqrt_of(ssg[:, 0:1], mg_t)
        for c in range(0, 4):
            transp_copy(c, nc.vector if c in (1, 3) else nc.gpsimd)
        new_pm(0)
        main_mm(0, 0, blk0[1])
        exp_acc(0, 0, blk0[1], 0)

        # GPSIMD: e-sq 4-7, then 8+, then g-sq rest (emission order)
        for c in range(4, min(8, NC)):
            square_acc(nc.gpsimd, en_src(c), ssn[:, c : c + 1])
        for c in range(8, NC):
            square_acc(nc.gpsimd, en_src(c), ssn[:, c : c + 1])
        if PC > 1:
            for c in range(1, PC):
                square_acc(nc.gpsimd, GbR[:, c - 1, :], ssg[:, c : c + 1])

        if NC > 4:
            rsqrt_of(ssn[:, 4 : min(8, NC)], ln8_t)
            for c in range(4, min(8, NC)):
                transp_copy(c, nc.vector if c in (5,) else nc.gpsimd)
            new_pm(1)
            main_mm(1, 0, 512)
            if NB > 1:
                main_mm(0, blk0[1], blk0[2])
                exp_acc(0, blk0[1], blk0[2], 1)
        if NC > 8:
            rsqrt_of(ssn[:, 8:NC], ln8_t)
            for c in range(8, NC):
                transp_copy(c, nc.gpsimd)
        if PC > 1:
            rsqrt_of(ssg[:, 1:PC], mg_t)
        if NB > 2:
            main_mm(0, blk0[2], blk0[3])
            exp_acc(0, blk0[2], blk0[3], 2)

        # diag-path squares/dots on DVE (run during the exp phase)
        dgots = small.tile([128, PC], f32)
        for c in range(PC):
            gsrc = Gb0[:, 0, :] if c == 0 else GbR[:, c - 1, :]
            dsq = scratch.tile([128, H], bf16, tag="dsq")
            nc.vector.scalar_tensor_tensor(
                out=dsq[:], in0=gsrc, scalar=1.0, in1=Ef[:, c, :],
                op0=OP.mult, op1=OP.mult, accum_out=dgots[:, c : c + 1],
            )
        for c in range(PC):
            square_acc(nc.vector, Ef[:, c, :], sse[:, c : c + 1])

        # chunk 1..: full-width matmuls + single exp each, with the remaining
        # ACT table ops slotted between exps at the point they become ready.
        for c in range(1, PC):
            if c not in pm_tiles:
                new_pm(c)
            for lo, hi in slabs:
                if c == 1 and lo == 0:
                    continue  # emitted early above
                main_mm(c, lo, hi)
            if c == 2:
                rsqrt_of(sse[:], ln64_t)  # sse <- 64*rsqrt(sse)
            if c == 3:
                # diag and its exp (dgots ready well before)
                diag = small.tile([128, PC], f32)
                nc.vector.tensor_mul(diag[:], dgots[:], ssg[:])
                nc.vector.tensor_mul(diag[:], diag[:], sse[:])
                ed = small.tile([128, PC], f32)
                nc.scalar.activation(
                    out=ed[:], in_=diag[:], func=AF.Exp, bias=cneg_t[:, 0:1], scale=1.0
                )
            exp_acc(c, 0, N1, NB - 1 + c)
        if PC <= 3:
            diag = small.tile([128, PC], f32)
            nc.vector.tensor_mul(diag[:], dgots[:], ssg[:])
            nc.vector.tensor_mul(diag[:], diag[:], sse[:])
            ed = small.tile([128, PC], f32)
            nc.scalar.activation(
                out=ed[:], in_=diag[:], func=AF.Exp, bias=cneg_t[:, 0:1], scale=1.0
            )

        # fold chunk-0 block partials (DVE; waits on exp0 accums)
        for i in range(1, NB):
            nc.vector.tensor_tensor(
                out=S[:, i : i + 1], in0=S[:, i - 1 : i], in1=S[:, i : i + 1], op=OP.add
            )

        # ---- per-row loss and masked partial sum
        t2 = small.tile([128, PC], f32)
        nc.vector.scalar_tensor_tensor(
            out=t2[:], in0=S[:, NB - 1 : NB - 1 + PC], scalar=corr_t[:, 0:1], in1=ed[:],
            op0=OP.add, op1=OP.add,
        )
        nc.scalar.activation(out=t2[:], in_=t2[:], func=AF.Ln, bias=eps_t[:, 0:1], scale=1.0)
        loss = small.tile([128, PC], f32)
        nc.vector.scalar_tensor_tensor(
            out=loss[:], in0=t2[:], scalar=CMAX, in1=diag[:],
            op0=OP.add, op1=OP.subtract,
        )
        lm = small.tile([128, PC], f32)
        part = small.tile([128, 1], f32)
        nc.vector.scalar_tensor_tensor(
            out=lm[:], in0=loss[:], scalar=1.0, in1=wt[:],
            op0=OP.mult, op1=OP.mult, accum_out=part[:],
        )
        nc.sync.dma_start(out=out[:], in_=part[:])
    if legalize:
        _legalize_waits(nc, max_waits=1)
    return nc


def _pad_rows(x: np.ndarray, n: int) -> np.ndarray:
    outp = np.zeros((n,) + x.shape[1:], dtype=x.dtype)
    outp[: x.shape[0]] = x
    return outp


def kernel(greek_embeds, english_embeds, labels):
    global LAST_RESULTS, LAST_SHAPES
    g = np.ascontiguousarray(np.asarray(greek_embeds, dtype=np.float32))
    e = np.ascontiguousarray(np.asarray(english_embeds, dtype=np.float32))
    lab = np.asarray(labels)
    B, P, Hh = g.shape
    assert Hh == H and B * 2 == N_CORES

    valid = lab != IGNORE_INDEX
    pos = valid & (lab == 1)
    neg = valid & (lab != 1)
    ok = (valid.sum(-1) >= 2) & pos.any(-1) & neg.any(-1)

    count = int(pos[ok].sum()) if ok.any() else 0
    if count == 0:
        return np.float32(0.0)

    pos_idx = [np.nonzero(pos[b])[0] if ok[b] else np.zeros(0, np.int64) for b in range(B)]
    neg_idx = [np.nonzero(neg[b])[0] if ok[b] else np.zeros(0, np.int64) for b in range(B)]
    halves = [np.array_split(pi, 2) for pi in pos_idx]

    np_max = max(len(halves[b][h]) for b in range(B) for h in range(2))
    nn_max = max(len(ni) for ni in neg_idx)
    P1 = max(128, ((np_max + 127) // 128) * 128)
    N1 = max(512, ((nn_max + 127) // 128) * 128)
    PC, NC = P1 // 128, N1 // 128

    E15 = np.float32(np.exp(np.float32(-CMAX)))
    bf16 = ml_dtypes.bfloat16
    fp8 = ml_dtypes.float8_e4m3
    in_maps = []
    for core in range(N_CORES):
        b, hf = core // 2, core % 2
        p_idx = halves[b][hf]
        n_idx = neg_idx[b]
        gr = _pad_rows(g[b][p_idx], P1)
        er = _pad_rows(e[b][n_idx], N1)
        epr = _pad_rows(e[b][p_idx], P1)
        w = np.zeros((128, PC), np.float32)
        npos = len(p_idx)
        for c in range(PC):
            w[: max(0, min(128, npos - c * 128)), c] = 1.0
        in_maps.append(
            {
                "g8t": np.ascontiguousarray(
                    (gr * FP8_SCALE).reshape(P1, 2, 128).transpose(2, 1, 0)
                ).astype(fp8),
                "gb": np.ascontiguousarray(
                    gr.astype(bf16).reshape(PC, 128, H).transpose(1, 0, 2)
                ),
                "ep": np.ascontiguousarray(
                    epr.astype(bf16).reshape(PC, 128, H).transpose(1, 0, 2)
                ),
                "en": np.ascontiguousarray(
                    er.astype(bf16).reshape(NC, 128, H).transpose(1, 0, 2)
                ),
                "wv": w,
                "corr": np.array([[-(N1 - len(n_idx)) * float(E15)]], np.float32),
            }
        )

    LAST_SHAPES = (P1, N1, dict(in_maps[0]))
    nc = _build_program(P1, N1)
    res = run_bass_kernel_spmd(nc, in_maps, list(range(N_CORES)), trace=TRACE)
    LAST_RESULTS = res
    total = sum(float(r["out"].sum()) for r in res.results)
    return np.float32(total / count)
